# revision 59
# baseline (speedup 1.0000x reference)
"""Trainium2 Bass kernel: PaLM-style parallel attention + FF transformer block.

Tensor-parallel over 8 NeuronCores: each core owns 2 heads (128 q/k/v cols of
W_in), 512 FF cols, and the matching 640 rows of W_out.  Each core computes a
full-shape partial output (bf16); the host sums the 8 partials (row-parallel
W_out).

v2: all matmuls in bf16 (2.4 GHz PE clock vs 1.2 GHz for fp32r), xbar
DMA-transposes instead of PE transposes, x fed as bf16 and read once per
batch, LN stats software-pipelined into the in-projection, attention
score->prob->PV chain software-pipelined so the PE never waits on the Act
engine, evictions spread across DVE/GpSimd, output written as bf16 partials.

Per-core dataflow (K-contractions on partitions):
  per 512-token chunk: x tiles (bf16) -> bn_stats -> mu/rs -> xn (bf16)
    -> xbar-transpose -> xnT [d, t]
  hT = W_slice^T @ xnT   (q,k rope-fused eviction; v re-transposed token-major
                          via xbar next to ones columns; ff -> gelu)
  per (i-chunk, head): ST[j,i] = kT^T q, PT = exp(0.125*ST) * causal_mask,
                       OT[0:64]=V^T PT accum, OT[64:128]=col-sums (ones cols)
  out_partial = [oT; ffgT]^T @ Wo_slice   (bf16 full-shape, host-summed)
"""

import numpy as np

HEADS = 16
HEAD_DIM = 64
HIDDEN = 1024
EXPF = 4
B = 2
L = 2048
NCORES = 8
HPC = HEADS // NCORES            # heads per core = 2
QS = HPC * HEAD_DIM              # per-core q/k/v width = 128
FFS = EXPF * HIDDEN // NCORES    # per-core ff width = 512
WSL = 3 * QS + FFS               # per-core W_in slice width = 896
KOUT = HIDDEN // 128             # 8 k-subtiles for hidden contraction
WOK = (QS + FFS) // 128          # 5 k-subtiles for out-proj contraction
LN_EPS = 1e-5

LAST_RESULTS = None  # BassKernelResults of the most recent HW run (for test.py)


# ----------------------------------------------------------------------------
# program builder
# ----------------------------------------------------------------------------

def build_program(b=B, l=L, debug=False, sim_gelu=False, opts=None,
                  dump=False):
    import concourse.bass as bass
    import concourse.mybir as mybir
    import concourse.tile as tile
    from concourse import bacc

    T = b * l
    NT = l // 128      # 128-token tiles per batch
    NC = l // 512      # 512-token chunks per batch
    f32 = mybir.dt.float32
    bf16 = mybir.dt.bfloat16
    AF = mybir.ActivationFunctionType
    OP = mybir.AluOpType

    opts = {**(opts or {})}
    nc = bacc.Bacc("TRN2", target_bir_lowering=False, debug=debug)

    x_d = nc.declare_dram_parameter("x", [T, HIDDEN], bf16, isOutput=False)
    w_d = nc.declare_dram_parameter("w_in", [128, KOUT, WSL], bf16, isOutput=False)
    wo_d = nc.declare_dram_parameter("w_out", [128, WOK, HIDDEN], bf16,
                                     isOutput=False)
    hb_d = nc.declare_dram_parameter("h_bias", [128, WSL // 128], f32,
                                     isOutput=False)
    cos_d = nc.declare_dram_parameter("cos_t", [128, l], bf16, isOutput=False)
    sinm_d = nc.declare_dram_parameter("sinm_t", [128, l], bf16, isOutput=False)
    mask_d = nc.declare_dram_parameter("mask_t", [128, 896], bf16, isOutput=False)
    out_d = nc.declare_dram_parameter("out", [T, HIDDEN], bf16, isOutput=True)
    if dump:
        dbg = {
            "d_xnT": nc.declare_dram_parameter(
                "d_xnT", [b, l // 512, 128, 4, KOUT, 128], bf16,
                isOutput=True),
            "d_qT": nc.declare_dram_parameter(
                "d_qT", [b, 128, l], bf16, isOutput=True),
            "d_kT": nc.declare_dram_parameter(
                "d_kT", [b, 128, l], bf16, isOutput=True),
            "d_vext": nc.declare_dram_parameter(
                "d_vext", [b, 128, l // 128, 256], bf16, isOutput=True),
            "d_ffgT": nc.declare_dram_parameter(
                "d_ffgT", [b, 128, EXPF, l], bf16, isOutput=True),
            "d_oT": nc.declare_dram_parameter(
                "d_oT", [b, 128, l], bf16, isOutput=True),
            "d_murs": nc.declare_dram_parameter(
                "d_murs", [2, 128, b * (l // 128)], f32, isOutput=True),
        }

    with tile.TileContext(nc) as tc:
        from contextlib import ExitStack
        with ExitStack() as ctx:
            const = ctx.enter_context(tc.tile_pool(name="const", bufs=1))
            strips = ctx.enter_context(tc.tile_pool(name="strips", bufs=1))
            xpool = ctx.enter_context(tc.tile_pool(name="xpool", bufs=12))
            stash = ctx.enter_context(tc.tile_pool(name="stash",
                                                   bufs=max(1, l // 512)))
            stats = ctx.enter_context(tc.tile_pool(name="stats", bufs=4))
            xnpool = ctx.enter_context(tc.tile_pool(name="xnpool", bufs=4))
            xntp = ctx.enter_context(tc.tile_pool(name="xntp", bufs=2))
            work = ctx.enter_context(tc.tile_pool(name="work", bufs=2))
            vtpool = ctx.enter_context(tc.tile_pool(name="vtpool", bufs=2))
            ptp = ctx.enter_context(tc.tile_pool(name="ptp", bufs=3))
            sums_p = ctx.enter_context(tc.tile_pool(name="sums", bufs=2))
            obuf = ctx.enter_context(tc.tile_pool(name="obuf", bufs=2))
            psA = ctx.enter_context(tc.tile_pool(name="psA", bufs=4,
                                                 space="PSUM"))
            psS = ctx.enter_context(tc.tile_pool(name="psS", bufs=2,
                                                 space="PSUM"))

            # batch-0 chunk-0 x tiles first: their consumers gate the PE start
            x0_tiles = []
            for t4 in range(4):
                xt = xpool.tile([128, HIDDEN], bf16, tag="xt", name="xtb")
                nc.sync.dma_start(xt[:], x_d[t4 * 128:(t4 + 1) * 128, :])
                x0_tiles.append(xt)

            # constants (host pre-arranged; all contiguous single DMAs);
            # wo_sb and mask_sb loads are deferred past the startup window
            w_sb = const.tile([128, KOUT, WSL], bf16)
            nc.sync.dma_start(w_sb[:], w_d[:])
            cos_sb = const.tile([128, l], bf16)
            sinm_sb = const.tile([128, l], bf16)
            hb_sb = const.tile([128, WSL // 128], f32)
            wo_sb = const.tile([128, WOK, HIDDEN], bf16)
            mask_sb = const.tile([128, 896], bf16)
            eps_sb = const.tile([128, 1], f32)
            nc.vector.memset(eps_sb[:], LN_EPS)
            onesb_sb = const.tile([128, 1], bf16)
            nc.vector.memset(onesb_sb[:], 1.0)

            mu_all = const.tile([128, b * NT], f32, tag="mu_all")
            var_all = const.tile([128, b * NT], f32, tag="var_all")
            sd_all = const.tile([128, b * NT], f32, tag="sd_all")
            rs_all = const.tile([128, b * NT], f32, tag="rs_all")
            sumx_all = const.tile([128, b * NT], f32, tag="sumx_all")
            sumsq_all = const.tile([128, b * NT], f32, tag="sumsq_all")
            xstash_of = {}
            strips_of = {}

            def stats_tile(bi, tt, xt, on_act=False):
                # LN statistics for one 128-token tile.  on_act: free-dim
                # sums via the Act engine's accum_out (Square/Copy live in
                # every act table, so no table swaps); else DVE bn_stats.
                gt = bi * NT + tt
                if on_act:
                    scr = stats.tile([128, HIDDEN], bf16, tag="scr")
                    nc.scalar.activation(scr[:], xt[:], AF.Square,
                                         accum_out=sumsq_all[:, gt:gt + 1])
                    nc.scalar.activation(scr[:], xt[:], AF.Copy,
                                         accum_out=sumx_all[:, gt:gt + 1])
                    return
                st6 = stats.tile([128, 2, 6], f32, tag="st6")
                nc.vector.bn_stats(st6[:, 0, :], xt[:, 0:512])
                nc.vector.bn_stats(st6[:, 1, :], xt[:, 512:1024])
                mv = stats.tile([128, 2], f32, tag="mv")
                nc.vector.bn_aggr(mv[:], st6[:])
                nc.vector.tensor_copy(mu_all[:, gt:gt + 1], mv[:, 0:1])
                nc.vector.tensor_copy(var_all[:, gt:gt + 1], mv[:, 1:2])

            def rs_cols(c0, n, from_sums=False):
                # rs = 1/sqrt(var+eps) for stats columns [c0, c0+n)
                cs = slice(c0, c0 + n)
                if from_sums:
                    # mu = sumx/H; var = sumsq/H - mu^2  (tiny DVE combines)
                    nc.vector.tensor_scalar(
                        out=mu_all[:, cs], in0=sumx_all[:, cs],
                        scalar1=1.0 / HIDDEN, scalar2=None, op0=OP.mult)
                    m2 = stats.tile([128, n], f32, tag="m2")
                    nc.vector.tensor_mul(m2[:], mu_all[:, cs], mu_all[:, cs])
                    nc.vector.scalar_tensor_tensor(
                        var_all[:, cs], sumsq_all[:, cs], 1.0 / HIDDEN,
                        m2[:], OP.mult, OP.subtract)
                nc.scalar.activation(sd_all[:, cs], var_all[:, cs],
                                     AF.Sqrt, bias=eps_sb[:])
                nc.vector.reciprocal_approx_fast(rs_all[:, cs],
                                                 sd_all[:, cs])

            def phase_b(bi):
                r0b = bi * l
                mu_s = mu_all[:, bi * NT:(bi + 1) * NT]
                rs_s = rs_all[:, bi * NT:(bi + 1) * NT]
                qT = strips.tile([128, l], bf16, tag="qT")
                kT = strips.tile([128, l], bf16, tag="kT")
                ffgT = strips.tile([128, EXPF, l], bf16, tag="ffgT")
                oT = strips.tile([128, l], bf16, tag="oT")
                vext = strips.tile([128, NT, 256], bf16, tag="vext")
                strips_of[bi] = (qT, kT, ffgT, oT, vext)

                # v_ext ones columns (produce the softmax row sums in PV)
                nc.vector.tensor_copy(
                    vext[:, :, 64:128],
                    onesb_sb[:, :, None].to_broadcast([128, NT, 64]))
                nc.vector.tensor_copy(
                    vext[:, :, 192:256],
                    onesb_sb[:, :, None].to_broadcast([128, NT, 64]))

                xt_of = {}

                def load_chunk(cj):
                    if bi != 0 or cj >= NC or cj in xt_of:
                        return
                    if cj == 0:
                        xt_of[0] = x0_tiles
                        return
                    tiles = []
                    for t4 in range(4):
                        tt = cj * 4 + t4
                        xt = xpool.tile([128, HIDDEN], bf16, tag="xt",
                                        name="xtb")
                        nc.sync.dma_start(
                            xt[:], x_d[r0b + tt * 128: r0b + (tt + 1) * 128, :])
                        tiles.append(xt)
                    xt_of[cj] = tiles

                def stats_chunk(cj):
                    if bi != 0 or cj >= NC:
                        return
                    on_act = cj > 0
                    for t4 in range(4):
                        stats_tile(bi, cj * 4 + t4, xt_of[cj][t4],
                                   on_act=on_act)
                    rs_cols(bi * NT + cj * 4, 4, from_sums=on_act)

                if bi == 0:
                    load_chunk(0)
                    load_chunk(1)
                    stats_chunk(0)

                for ci in range(NC):
                    if bi == 0:
                        xts = xt_of.pop(ci)
                    else:
                        xst = xstash_of[bi][ci]
                        xts = [xst[:, t4, :] for t4 in range(4)]
                    # xnT4[d, t4, ko, t] = xn4[t, t4, ko*128+d] via one (or
                    # four, for the latency-critical first chunk) xbar DMAs
                    xnT4 = xntp.tile([128, 4, KOUT, 128], bf16, tag="xnT")
                    xn4 = xnpool.tile([128, 4, HIDDEN], bf16, tag="xn")
                    for t4 in range(4):
                        tt = ci * 4 + t4
                        nc.vector.tensor_scalar(
                            out=xn4[:, t4, :], in0=xts[t4][:],
                            scalar1=mu_s[:, tt:tt + 1],
                            scalar2=rs_s[:, tt:tt + 1],
                            op0=OP.subtract, op1=OP.mult)
                        if bi == 0 and ci == 0:
                            nc.sync.dma_start(
                                xnT4[:, t4, :, :], xn4[:, t4, :],
                                transpose=True)
                    if not (bi == 0 and ci == 0):
                        nc.sync.dma_start(
                            xnT4[:].rearrange("p a b t -> p (a b) t"),
                            xn4[:].rearrange("p a d -> p (a d)"),
                            transpose=True)
                    if bi == 0 and ci == 0:
                        # rope/bias consts: issued after the startup-critical
                        # chunk-0 transposes, needed only ~2us later
                        nc.sync.dma_start(cos_sb[:], cos_d[:])
                        nc.sync.dma_start(sinm_sb[:], sinm_d[:])
                        nc.sync.dma_start(hb_sb[:], hb_d[:])
                    if dump:
                        nc.sync.dma_start(dbg["d_xnT"][bi, ci], xnT4[:])
                    for m in range(3 + EXPF):
                        hps = psA.tile([128, 512], f32, tag="p512", name="hps")
                        for ko in range(KOUT):
                            nc.tensor.matmul(
                                hps[:],
                                w_sb[:, ko, m * 128:(m + 1) * 128],
                                xnT4[:, :, ko, :],
                                start=(ko == 0), stop=(ko == KOUT - 1))
                        if m < 2:
                            # q/k: rope-fused eviction (+ h bias)
                            dst = (qT if m == 0 else kT)[:, ci * 512:(ci + 1) * 512]
                            cs = cos_sb[:, ci * 512:(ci + 1) * 512]
                            sn = sinm_sb[:, ci * 512:(ci + 1) * 512]
                            tmpc = work.tile([128, 512], f32, tag="rtmp1")
                            t2 = work.tile([128, 512], f32, tag="rtmp2")
                            nc.vector.scalar_tensor_tensor(
                                tmpc[:], hps[:], hb_sb[:, m:m + 1], cs,
                                OP.add, OP.mult)
                            for h in range(HPC):
                                r0 = h * 64
                                nc.vector.scalar_tensor_tensor(
                                    t2[r0:r0 + 32, :], hps[r0 + 32:r0 + 64, :],
                                    hb_sb[r0 + 32:r0 + 64, m:m + 1],
                                    sn[r0 + 32:r0 + 64, :], OP.add, OP.mult)
                                nc.vector.scalar_tensor_tensor(
                                    t2[r0 + 32:r0 + 64, :], hps[r0:r0 + 32, :],
                                    hb_sb[r0:r0 + 32, m:m + 1],
                                    sn[r0:r0 + 32, :], OP.add, OP.mult)
                            nc.vector.tensor_add(dst, tmpc[:], t2[:])
                        elif m == 2:
                            # v: bias, then xbar re-transpose to token-major
                            vtmp = vtpool.tile([128, 512], bf16, tag="vtmp")
                            nc.vector.tensor_scalar_add(
                                vtmp[:], hps[:], hb_sb[:, 2:3])
                            # vext[t, ci*4+o, 0:64]    = v_h0[t]
                            # vext[t, ci*4+o, 128:192] = v_h1[t]
                            nc.sync.dma_start_transpose(
                                vext[:, ci * 4:(ci + 1) * 4, 0:64],
                                vtmp[0:64, :])
                            nc.sync.dma_start_transpose(
                                vext[:, ci * 4:(ci + 1) * 4, 128:192],
                                vtmp[64:128, :])
                            # lookahead loads + stats (Act-engine sums) go here
                            load_chunk(ci + 2)
                            stats_chunk(ci + 1)
                        else:
                            nc.scalar.activation(
                                ffgT[:, m - 3, ci * 512:(ci + 1) * 512], hps[:],
                                AF.Identity if sim_gelu else AF.Gelu,
                                bias=hb_sb[:, m:m + 1])
                if dump:
                    nc.sync.dma_start(dbg["d_qT"][bi], qT[:])
                    nc.sync.dma_start(dbg["d_kT"][bi], kT[:])
                    nc.sync.dma_start(dbg["d_vext"][bi], vext[:])
                    nc.sync.dma_start(dbg["d_ffgT"][bi], ffgT[:])
                    nc.sync.dma_start(dbg["d_murs"][0], mu_all[:])
                    nc.sync.dma_start(dbg["d_murs"][1], rs_all[:])

            def phase_c(bi):
                qT, kT, ffgT, oT, vext = strips_of[bi]
                for ic in range(NC):
                    ot = [psA.tile([128, 512], f32, tag="p512",
                                   name=f"ot{_h}")
                          for _h in range(HPC)]
                    njt = (ic + 1) * 4
                    pts = {}
                    for step in range(njt + 1):
                        if step < njt:
                            jt = step
                            st2 = psS.tile([128, 1024], f32, tag="st2")
                            for h in range(HPC):
                                nc.tensor.matmul(
                                    st2[:, h * 512:(h + 1) * 512],
                                    kT[h * 64:(h + 1) * 64,
                                       jt * 128:(jt + 1) * 128],
                                    qT[h * 64:(h + 1) * 64,
                                       ic * 512:(ic + 1) * 512],
                                    start=True, stop=True)
                            pt2 = ptp.tile([128, 1024], bf16, tag="pt")
                            nc.scalar.activation(
                                pt2[:], st2[:], AF.Exp,
                                scale=float(HEAD_DIM) ** -0.5)
                            d = jt * 128 - ic * 512
                            if d >= 0:
                                nc.vector.tensor_tensor(
                                    pt2[:].rearrange("p (g c) -> p g c", c=512),
                                    pt2[:].rearrange("p (g c) -> p g c", c=512),
                                    mask_sb[:, None, 384 - d:896 - d]
                                    .to_broadcast([128, HPC, 512]),
                                    OP.mult)
                            pts[jt] = pt2
                        if step >= 1:
                            jt = step - 1
                            pt2 = pts.pop(jt)
                            for h in range(HPC):
                                nc.tensor.matmul(
                                    ot[h][:],
                                    vext[:, jt, h * 128:(h + 1) * 128],
                                    pt2[:, h * 512:(h + 1) * 512],
                                    start=(jt == 0), stop=(jt == njt - 1))
                    for h in range(HPC):
                        sums_sb = sums_p.tile([64, 512], f32, tag="sums")
                        # approx recip needs an SBUF source (PSUM reads are
                        # silently wrong on HW for custom DVE ops)
                        nc.vector.tensor_copy(sums_sb[:], ot[h][64:128, :])
                        nc.vector.reciprocal_approx_fast(
                            sums_sb[:], sums_sb[:])
                        nc.vector.tensor_mul(
                            oT[h * 64:(h + 1) * 64, ic * 512:(ic + 1) * 512],
                            ot[h][0:64, :], sums_sb[:])
                if dump:
                    nc.sync.dma_start(dbg["d_oT"][bi], oT[:])

            def phase_a1(bi):
                # batch-bi x stash + LN stats, hidden under C/D.  One tile
                # per chunk: per-tile DMA gating avoids completion-order races.
                r0b = bi * l
                xstash_of[bi] = {}
                for cj in range(NC):
                    xst = stash.tile([128, 4, HIDDEN], bf16, tag="xst",
                                     name=f"xst{bi}_{cj}")
                    xstash_of[bi][cj] = xst
                    nc.sync.dma_start(
                        xst[:],
                        x_d[r0b + cj * 512:r0b + (cj + 1) * 512, :]
                        .rearrange("(o p) d -> p o d", p=128))
                    for t4 in range(4):
                        stats_tile(bi, cj * 4 + t4, xst[:, t4, :])
                    rs_cols(bi * NT + cj * 4, 4)

            def phase_d(bi):
                r0b = bi * l
                qT, kT, ffgT, oT, vext = strips_of[bi]
                for tt in range(NT):
                    ob = obuf.tile([128, HIDDEN], bf16, tag="ob")
                    for n2 in range(2):
                        ops = psA.tile([128, 512], f32, tag="p512", name="ops")
                        nc.tensor.matmul(
                            ops[:], oT[:, tt * 128:(tt + 1) * 128],
                            wo_sb[:, 0, n2 * 512:(n2 + 1) * 512],
                            start=True, stop=False)
                        for kk in range(EXPF):
                            nc.tensor.matmul(
                                ops[:], ffgT[:, kk, tt * 128:(tt + 1) * 128],
                                wo_sb[:, kk + 1, n2 * 512:(n2 + 1) * 512],
                                start=False, stop=(kk == EXPF - 1))
                        nc.scalar.activation(
                            ob[:, n2 * 512:(n2 + 1) * 512], ops[:], AF.Copy)
                    nc.sync.dma_start(
                        out_d[r0b + tt * 128: r0b + (tt + 1) * 128, :], ob[:])

            phase_b(0)
            nc.sync.dma_start(mask_sb[:], mask_d[:])
            phase_c(0)
            nc.sync.dma_start(wo_sb[:], wo_d[:])
            if b > 1:
                phase_a1(1)
            phase_d(0)
            for bi in range(1, b):
                phase_b(bi)
                phase_c(bi)
                phase_d(bi)

    nc.compile()
    return nc


# ----------------------------------------------------------------------------
# host-side constants and per-core input slicing
# ----------------------------------------------------------------------------

def _rope_tables(l):
    inv_freq = 1.0 / (10000.0 ** (np.arange(0, HEAD_DIM, 2, dtype=np.float32)
                                  / HEAD_DIM))                       # [32]
    t = np.arange(l, dtype=np.float32)
    fr = t[None, :] * inv_freq[:, None]                              # [32, l]
    cos1 = np.cos(np.concatenate([fr, fr], axis=0))                  # [64, l]
    sin1 = np.sin(np.concatenate([fr, fr], axis=0))                  # [64, l]
    sinm1 = np.concatenate([-sin1[:32], sin1[32:]], axis=0)          # sign-folded
    # half-swapped so the stt source base partition matches the operand rows
    sinswap1 = np.concatenate([sinm1[32:], sinm1[:32]], axis=0)
    cos = np.tile(cos1, (HPC, 1)).astype(np.float32)                 # [128, l]
    sinswap = np.tile(sinswap1, (HPC, 1)).astype(np.float32)
    return cos, sinswap


def _mask_strip():
    # strip[r, u] = 1 iff u >= r + 384; diagonal block at offset d uses
    # cols [384-d : 896-d] so that mask[r, c] = (c >= r + d)
    r = np.arange(128)[:, None]
    u = np.arange(896)[None, :]
    return (u >= r + 384).astype(np.float32)


def core_inputs(x_bf, ln_w, ln_b, W_in, W_out, c, l=L):
    """Build the per-core input map for core c (pure numpy).

    x_bf: [T, HIDDEN] bf16 (pre-cast once by the caller)."""
    import ml_dtypes
    bf16 = ml_dtypes.bfloat16
    ln_w = np.asarray(ln_w, np.float32)
    ln_b = np.asarray(ln_b, np.float32)
    W_in = np.asarray(W_in, np.float32)
    W_out = np.asarray(W_out, np.float32)

    qc = slice(c * QS, (c + 1) * QS)
    kc = slice(HIDDEN + c * QS, HIDDEN + (c + 1) * QS)
    vc = slice(2 * HIDDEN + c * QS, 2 * HIDDEN + (c + 1) * QS)
    fc = slice(3 * HIDDEN + c * FFS, 3 * HIDDEN + (c + 1) * FFS)
    w_raw = np.concatenate(
        [W_in[:, qc], W_in[:, kc], W_in[:, vc], W_in[:, fc]], axis=1)  # [1024, 896]
    w_slice = w_raw * ln_w[:, None]
    # device layout [128, KOUT, WSL]: w_arr[p, o, f] = w_slice[o*128+p, f]
    w_arr = np.ascontiguousarray(
        w_slice.reshape(KOUT, 128, WSL).transpose(1, 0, 2).astype(bf16))
    h_bias = (ln_b @ w_raw)                                            # [896]
    hb_arr = np.ascontiguousarray(
        h_bias.reshape(WSL // 128, 128).T.astype(np.float32))          # [128, 7]
    wo_slice = np.concatenate(
        [W_out[c * QS:(c + 1) * QS, :],
         W_out[HIDDEN + c * FFS: HIDDEN + (c + 1) * FFS, :]], axis=0)  # [640, 1024]
    wo_arr = np.ascontiguousarray(
        wo_slice.reshape(WOK, 128, HIDDEN).transpose(1, 0, 2).astype(bf16))

    cos, sinm = _rope_tables(l)
    return {
        "x": x_bf,
        "w_in": w_arr,
        "w_out": wo_arr,
        "h_bias": hb_arr,
        "cos_t": np.ascontiguousarray(cos.astype(bf16)),
        "sinm_t": np.ascontiguousarray(sinm.astype(bf16)),
        "mask_t": np.ascontiguousarray(_mask_strip().astype(bf16)),
    }


# ----------------------------------------------------------------------------
# entry point
# ----------------------------------------------------------------------------

_PROG_CACHE = {}


def kernel(x, ln_w, ln_b, W_in, W_out):
    global LAST_RESULTS
    import ml_dtypes
    from concourse import bass_utils
    from concourse.bass_interp import get_hw_module

    x = np.asarray(x, np.float32)
    b, l = x.shape[0], x.shape[1]

    key = (b, l)
    if key not in _PROG_CACHE:
        _PROG_CACHE[key] = build_program(b=b, l=l, debug=False)
    nc = _PROG_CACHE[key]

    x_bf = np.ascontiguousarray(
        x.reshape(b * l, HIDDEN).astype(ml_dtypes.bfloat16))
    in_maps = [core_inputs(x_bf, ln_w, ln_b, W_in, W_out, c, l=l)
               for c in range(NCORES)]

    old_m = nc.m
    nc.m = get_hw_module(nc.m)
    try:
        res = bass_utils.run_bass_kernel_spmd(
            nc, in_maps, core_ids=list(range(NCORES)),
            trace=bool(int(__import__("os").environ.get("BASS_TRACE_RUN", "0"))))
    finally:
        nc.m = old_m
    LAST_RESULTS = res

    acc = np.zeros((b * l, HIDDEN), np.float64)
    for r in res.results:
        acc += r["out"].astype(np.float64)
    return acc.reshape(b, l, HIDDEN).astype(np.float32)


# revision 68
# speedup vs baseline: 1.1761x; 1.1761x over previous
"""Trainium2 Bass kernel: PaLM-style parallel attention + FF transformer block.

Tensor-parallel over 8 NeuronCores: each core owns 2 heads (128 q/k/v cols of
W_in), 512 FF cols, and the matching 640 rows of W_out.  Each core computes a
full-shape partial output (bf16); the host sums the 8 partials (row-parallel
W_out).

v2: all matmuls in bf16 (2.4 GHz PE clock vs 1.2 GHz for fp32r), xbar
DMA-transposes instead of PE transposes, x fed as bf16 and read once per
batch, LN stats software-pipelined into the in-projection, attention
score->prob->PV chain software-pipelined so the PE never waits on the Act
engine, evictions spread across DVE/GpSimd, output written as bf16 partials.

Per-core dataflow (K-contractions on partitions):
  per 512-token chunk: x tiles (bf16) -> bn_stats -> mu/rs -> xn (bf16)
    -> xbar-transpose -> xnT [d, t]
  hT = W_slice^T @ xnT   (q,k rope-fused eviction; v re-transposed token-major
                          via xbar next to ones columns; ff -> gelu)
  per (i-chunk, head): ST[j,i] = kT^T q, PT = exp(0.125*ST) * causal_mask,
                       OT[0:64]=V^T PT accum, OT[64:128]=col-sums (ones cols)
  out_partial = [oT; ffgT]^T @ Wo_slice   (bf16 full-shape, host-summed)
"""

import numpy as np

HEADS = 16
HEAD_DIM = 64
HIDDEN = 1024
EXPF = 4
B = 2
L = 2048
NCORES = 8
HPC = HEADS // NCORES            # heads per core = 2
QS = HPC * HEAD_DIM              # per-core q/k/v width = 128
FFS = EXPF * HIDDEN // NCORES    # per-core ff width = 512
WSL = 3 * QS + FFS               # per-core W_in slice width = 896
KOUT = HIDDEN // 128             # 8 k-subtiles for hidden contraction
WOK = (QS + FFS) // 128          # 5 k-subtiles for out-proj contraction
LN_EPS = 1e-5

LAST_RESULTS = None  # BassKernelResults of the most recent HW run (for test.py)


# ----------------------------------------------------------------------------
# program builder
# ----------------------------------------------------------------------------

def build_program(b=B, l=L, debug=False, sim_gelu=False, opts=None,
                  dump=False):
    import concourse.bass as bass
    import concourse.mybir as mybir
    import concourse.tile as tile
    from concourse import bacc

    T = b * l
    NT = l // 128      # 128-token tiles per batch
    NC = l // 512      # 512-token chunks per batch
    f32 = mybir.dt.float32
    bf16 = mybir.dt.bfloat16
    AF = mybir.ActivationFunctionType
    OP = mybir.AluOpType

    opts = {**(opts or {})}
    nc = bacc.Bacc("TRN2", target_bir_lowering=False, debug=debug)

    x_d = nc.declare_dram_parameter("x", [T, HIDDEN], bf16, isOutput=False)
    w_d = nc.declare_dram_parameter("w_in", [128, KOUT, WSL], bf16, isOutput=False)
    wo_d = nc.declare_dram_parameter("w_out", [128, WOK, HIDDEN], bf16,
                                     isOutput=False)
    hb_d = nc.declare_dram_parameter("h_bias", [128, WSL // 128], f32,
                                     isOutput=False)
    cos_d = nc.declare_dram_parameter("cos_t", [128, l], bf16, isOutput=False)
    sinm_d = nc.declare_dram_parameter("sinm_t", [128, l], bf16, isOutput=False)
    mask_d = nc.declare_dram_parameter("mask_t", [128, 896], bf16, isOutput=False)
    out_d = nc.declare_dram_parameter("out", [T, HIDDEN], bf16, isOutput=True)
    if dump:
        dbg = {
            "d_xnT": nc.declare_dram_parameter(
                "d_xnT", [b, l // 512, 128, 4, KOUT, 128], bf16,
                isOutput=True),
            "d_qT": nc.declare_dram_parameter(
                "d_qT", [b, 128, l], bf16, isOutput=True),
            "d_kT": nc.declare_dram_parameter(
                "d_kT", [b, 128, l], bf16, isOutput=True),
            "d_vext": nc.declare_dram_parameter(
                "d_vext", [b, 128, l // 128, 256], bf16, isOutput=True),
            "d_ffgT": nc.declare_dram_parameter(
                "d_ffgT", [b, 128, EXPF, l], bf16, isOutput=True),
            "d_oT": nc.declare_dram_parameter(
                "d_oT", [b, 128, l], bf16, isOutput=True),
            "d_murs": nc.declare_dram_parameter(
                "d_murs", [2, 128, b * (l // 128)], f32, isOutput=True),
        }

    with tile.TileContext(nc) as tc:
        from contextlib import ExitStack
        with ExitStack() as ctx:
            const = ctx.enter_context(tc.tile_pool(name="const", bufs=1))
            strips = ctx.enter_context(tc.tile_pool(name="strips", bufs=1))
            xpool = ctx.enter_context(tc.tile_pool(name="xpool", bufs=12))
            stash = ctx.enter_context(tc.tile_pool(name="stash", bufs=1))
            stats = ctx.enter_context(tc.tile_pool(name="stats", bufs=4))
            xnpool = ctx.enter_context(tc.tile_pool(name="xnpool", bufs=4))
            xntp = ctx.enter_context(tc.tile_pool(name="xntp", bufs=2))
            work = ctx.enter_context(tc.tile_pool(name="work", bufs=2))
            vtpool = ctx.enter_context(tc.tile_pool(name="vtpool", bufs=2))
            ptp = ctx.enter_context(tc.tile_pool(name="ptp", bufs=3))
            sums_p = ctx.enter_context(tc.tile_pool(name="sums", bufs=2))
            obuf = ctx.enter_context(tc.tile_pool(name="obuf", bufs=2))
            psA = ctx.enter_context(tc.tile_pool(name="psA", bufs=4,
                                                 space="PSUM"))
            psS = ctx.enter_context(tc.tile_pool(name="psS", bufs=2,
                                                 space="PSUM"))

            # batch-0 chunk-0 x tiles first: their consumers gate the PE start
            x0_tiles = []
            for t4 in range(4):
                xt = xpool.tile([128, HIDDEN], bf16, tag="xt", name="xtb")
                nc.sync.dma_start(xt[:], x_d[t4 * 128:(t4 + 1) * 128, :])
                x0_tiles.append(xt)

            # constants (host pre-arranged; all contiguous single DMAs);
            # wo_sb and mask_sb loads are deferred past the startup window
            w_sb = const.tile([128, KOUT, WSL], bf16)
            nc.sync.dma_start(w_sb[:], w_d[:])
            cos_sb = const.tile([128, l], bf16)
            nc.sync.dma_start(cos_sb[:], cos_d[:])
            sinm_sb = const.tile([128, l], bf16)
            nc.sync.dma_start(sinm_sb[:], sinm_d[:])
            hb_sb = const.tile([128, WSL // 128], f32)
            nc.sync.dma_start(hb_sb[:], hb_d[:])
            wo_sb = const.tile([128, WOK, HIDDEN], bf16)
            mask_sb = const.tile([128, 896], bf16)
            eps_sb = const.tile([128, 1], f32)
            nc.vector.memset(eps_sb[:], LN_EPS)
            onesb_sb = const.tile([128, 1], bf16)
            nc.vector.memset(onesb_sb[:], 1.0)

            mu_all = const.tile([128, b * NT], f32, tag="mu_all")
            var_all = const.tile([128, b * NT], f32, tag="var_all")
            sd_all = const.tile([128, b * NT], f32, tag="sd_all")
            rs_all = const.tile([128, b * NT], f32, tag="rs_all")
            xstash_of = {}
            strips_of = {}

            def stats_tile(bi, tt, xt):
                # LN statistics for one 128-token tile (DVE only)
                gt = bi * NT + tt
                st6 = stats.tile([128, 2, 6], f32, tag="st6")
                nc.vector.bn_stats(st6[:, 0, :], xt[:, 0:512])
                nc.vector.bn_stats(st6[:, 1, :], xt[:, 512:1024])
                mv = stats.tile([128, 2], f32, tag="mv")
                nc.vector.bn_aggr(mv[:], st6[:])
                nc.vector.tensor_copy(mu_all[:, gt:gt + 1], mv[:, 0:1])
                nc.vector.tensor_copy(var_all[:, gt:gt + 1], mv[:, 1:2])

            def rs_cols(c0, n):
                # rs = 1/sqrt(var+eps) for stats columns [c0, c0+n)
                cs = slice(c0, c0 + n)
                nc.scalar.activation(sd_all[:, cs], var_all[:, cs],
                                     AF.Sqrt, bias=eps_sb[:])
                nc.vector.reciprocal_approx_fast(rs_all[:, cs],
                                                 sd_all[:, cs])

            def phase_b(bi):
                r0b = bi * l
                mu_s = mu_all[:, bi * NT:(bi + 1) * NT]
                rs_s = rs_all[:, bi * NT:(bi + 1) * NT]
                qT = strips.tile([128, l], bf16, tag="qT")
                kT = strips.tile([128, l], bf16, tag="kT")
                ffgT = strips.tile([128, EXPF, l], bf16, tag="ffgT")
                oT = strips.tile([128, l], bf16, tag="oT")
                vext = strips.tile([128, NT, 256], bf16, tag="vext")
                strips_of[bi] = (qT, kT, ffgT, oT, vext)

                # v_ext ones columns (produce the softmax row sums in PV)
                nc.vector.tensor_copy(
                    vext[:, :, 64:128],
                    onesb_sb[:, :, None].to_broadcast([128, NT, 64]))
                nc.vector.tensor_copy(
                    vext[:, :, 192:256],
                    onesb_sb[:, :, None].to_broadcast([128, NT, 64]))

                xt_of = {}

                def load_chunk(cj):
                    if bi != 0 or cj >= NC or cj in xt_of:
                        return
                    if cj == 0:
                        xt_of[0] = x0_tiles
                        return
                    tiles = []
                    for t4 in range(4):
                        tt = cj * 4 + t4
                        xt = xpool.tile([128, HIDDEN], bf16, tag="xt",
                                        name="xtb")
                        nc.sync.dma_start(
                            xt[:], x_d[r0b + tt * 128: r0b + (tt + 1) * 128, :])
                        tiles.append(xt)
                    xt_of[cj] = tiles

                def stats_chunk(cj):
                    if bi != 0 or cj >= NC:
                        return
                    for t4 in range(4):
                        stats_tile(bi, cj * 4 + t4, xt_of[cj][t4])
                    rs_cols(bi * NT + cj * 4, 4)

                if bi == 0:
                    load_chunk(0)
                    load_chunk(1)
                    stats_chunk(0)

                for ci in range(NC):
                    if bi == 0:
                        xts = xt_of.pop(ci)
                    else:
                        xst = xstash_of[bi]
                        xts = [xst[:, ci * 4 + t4, :] for t4 in range(4)]
                    # xnT4[d, t4, ko, t] = xn4[t, t4, ko*128+d] via one (or
                    # four, for the latency-critical first chunk) xbar DMAs
                    xnT4 = xntp.tile([128, 4, KOUT, 128], bf16, tag="xnT")
                    xn4 = xnpool.tile([128, 4, HIDDEN], bf16, tag="xn")
                    for t4 in range(4):
                        tt = ci * 4 + t4
                        nc.vector.tensor_scalar(
                            out=xn4[:, t4, :], in0=xts[t4][:],
                            scalar1=mu_s[:, tt:tt + 1],
                            scalar2=rs_s[:, tt:tt + 1],
                            op0=OP.subtract, op1=OP.mult)
                        if bi == 0 and ci == 0:
                            nc.sync.dma_start(
                                xnT4[:, t4, :, :], xn4[:, t4, :],
                                transpose=True)
                    if not (bi == 0 and ci == 0):
                        nc.sync.dma_start(
                            xnT4[:].rearrange("p a b t -> p (a b) t"),
                            xn4[:].rearrange("p a d -> p (a d)"),
                            transpose=True)
                    if dump:
                        nc.sync.dma_start(dbg["d_xnT"][bi, ci], xnT4[:])
                    for m in range(3 + EXPF):
                        hps = psA.tile([128, 512], f32, tag="p512", name="hps")
                        for ko in range(KOUT):
                            nc.tensor.matmul(
                                hps[:],
                                w_sb[:, ko, m * 128:(m + 1) * 128],
                                xnT4[:, :, ko, :],
                                start=(ko == 0), stop=(ko == KOUT - 1))
                        if m < 2:
                            # q/k: rope-fused eviction (+ h bias)
                            dst = (qT if m == 0 else kT)[:, ci * 512:(ci + 1) * 512]
                            cs = cos_sb[:, ci * 512:(ci + 1) * 512]
                            sn = sinm_sb[:, ci * 512:(ci + 1) * 512]
                            tmpc = work.tile([128, 512], f32, tag="rtmp1")
                            t2 = work.tile([128, 512], f32, tag="rtmp2")
                            nc.vector.scalar_tensor_tensor(
                                tmpc[:], hps[:], hb_sb[:, m:m + 1], cs,
                                OP.add, OP.mult)
                            for h in range(HPC):
                                r0 = h * 64
                                nc.vector.scalar_tensor_tensor(
                                    t2[r0:r0 + 32, :], hps[r0 + 32:r0 + 64, :],
                                    hb_sb[r0 + 32:r0 + 64, m:m + 1],
                                    sn[r0 + 32:r0 + 64, :], OP.add, OP.mult)
                                nc.vector.scalar_tensor_tensor(
                                    t2[r0 + 32:r0 + 64, :], hps[r0:r0 + 32, :],
                                    hb_sb[r0:r0 + 32, m:m + 1],
                                    sn[r0:r0 + 32, :], OP.add, OP.mult)
                            nc.vector.tensor_add(dst, tmpc[:], t2[:])
                        elif m == 2:
                            # v: bias, then xbar re-transpose to token-major
                            vtmp = vtpool.tile([128, 512], bf16, tag="vtmp")
                            nc.vector.tensor_scalar_add(
                                vtmp[:], hps[:], hb_sb[:, 2:3])
                            # vext[t, ci*4+o, 0:64]    = v_h0[t]
                            # vext[t, ci*4+o, 128:192] = v_h1[t]
                            nc.sync.dma_start_transpose(
                                vext[:, ci * 4:(ci + 1) * 4, 0:64],
                                vtmp[0:64, :])
                            nc.sync.dma_start_transpose(
                                vext[:, ci * 4:(ci + 1) * 4, 128:192],
                                vtmp[64:128, :])
                            # lookahead loads + stats (Act-engine sums) go here
                            load_chunk(ci + 2)
                            stats_chunk(ci + 1)
                        else:
                            nc.scalar.activation(
                                ffgT[:, m - 3, ci * 512:(ci + 1) * 512], hps[:],
                                AF.Identity if sim_gelu else AF.Gelu,
                                bias=hb_sb[:, m:m + 1])
                if dump:
                    nc.sync.dma_start(dbg["d_qT"][bi], qT[:])
                    nc.sync.dma_start(dbg["d_kT"][bi], kT[:])
                    nc.sync.dma_start(dbg["d_vext"][bi], vext[:])
                    nc.sync.dma_start(dbg["d_ffgT"][bi], ffgT[:])
                    nc.sync.dma_start(dbg["d_murs"][0], mu_all[:])
                    nc.sync.dma_start(dbg["d_murs"][1], rs_all[:])

            def phase_c(bi):
                qT, kT, ffgT, oT, vext = strips_of[bi]
                for ic in range(NC):
                    ot = [psA.tile([128, 512], f32, tag="p512",
                                   name=f"ot{_h}")
                          for _h in range(HPC)]
                    njt = (ic + 1) * 4
                    pts = {}
                    for step in range(njt + 1):
                        if step < njt:
                            jt = step
                            st2 = psS.tile([128, 1024], f32, tag="st2")
                            for h in range(HPC):
                                nc.tensor.matmul(
                                    st2[:, h * 512:(h + 1) * 512],
                                    kT[h * 64:(h + 1) * 64,
                                       jt * 128:(jt + 1) * 128],
                                    qT[h * 64:(h + 1) * 64,
                                       ic * 512:(ic + 1) * 512],
                                    start=True, stop=True)
                            pt2 = ptp.tile([128, 1024], bf16, tag="pt")
                            nc.scalar.activation(
                                pt2[:], st2[:], AF.Exp,
                                scale=float(HEAD_DIM) ** -0.5)
                            d = jt * 128 - ic * 512
                            if d >= 0:
                                nc.vector.tensor_tensor(
                                    pt2[:].rearrange("p (g c) -> p g c", c=512),
                                    pt2[:].rearrange("p (g c) -> p g c", c=512),
                                    mask_sb[:, None, 384 - d:896 - d]
                                    .to_broadcast([128, HPC, 512]),
                                    OP.mult)
                            pts[jt] = pt2
                        if step >= 1:
                            jt = step - 1
                            pt2 = pts.pop(jt)
                            for h in range(HPC):
                                nc.tensor.matmul(
                                    ot[h][:],
                                    vext[:, jt, h * 128:(h + 1) * 128],
                                    pt2[:, h * 512:(h + 1) * 512],
                                    start=(jt == 0), stop=(jt == njt - 1))
                    for h in range(HPC):
                        sums_sb = sums_p.tile([64, 512], f32, tag="sums")
                        # approx recip needs an SBUF source (PSUM reads are
                        # silently wrong on HW for custom DVE ops)
                        nc.vector.tensor_copy(sums_sb[:], ot[h][64:128, :])
                        nc.vector.reciprocal_approx_fast(
                            sums_sb[:], sums_sb[:])
                        nc.vector.tensor_mul(
                            oT[h * 64:(h + 1) * 64, ic * 512:(ic + 1) * 512],
                            ot[h][0:64, :], sums_sb[:])
                if dump:
                    nc.sync.dma_start(dbg["d_oT"][bi], oT[:])

            def phase_a1(bi):
                # batch-bi x stash (one big DMA) + LN stats, hidden under C/D
                r0b = bi * l
                xst = stash.tile([128, NT, HIDDEN], bf16, tag="xst",
                                 name=f"xst{bi}")
                xstash_of[bi] = xst
                nc.sync.dma_start(
                    xst[:],
                    x_d[r0b:r0b + l, :].rearrange("(o p) d -> p o d", p=128))
                for tt in range(NT):
                    stats_tile(bi, tt, xst[:, tt, :])
                rs_cols(bi * NT, NT)

            def phase_d(bi):
                r0b = bi * l
                qT, kT, ffgT, oT, vext = strips_of[bi]
                for tt in range(NT):
                    ob = obuf.tile([128, HIDDEN], bf16, tag="ob")
                    for n2 in range(2):
                        ops = psA.tile([128, 512], f32, tag="p512", name="ops")
                        nc.tensor.matmul(
                            ops[:], oT[:, tt * 128:(tt + 1) * 128],
                            wo_sb[:, 0, n2 * 512:(n2 + 1) * 512],
                            start=True, stop=False)
                        for kk in range(EXPF):
                            nc.tensor.matmul(
                                ops[:], ffgT[:, kk, tt * 128:(tt + 1) * 128],
                                wo_sb[:, kk + 1, n2 * 512:(n2 + 1) * 512],
                                start=False, stop=(kk == EXPF - 1))
                        nc.scalar.activation(
                            ob[:, n2 * 512:(n2 + 1) * 512], ops[:], AF.Copy)
                    nc.sync.dma_start(
                        out_d[r0b + tt * 128: r0b + (tt + 1) * 128, :], ob[:])

            phase_b(0)
            nc.sync.dma_start(mask_sb[:], mask_d[:])
            phase_c(0)
            nc.sync.dma_start(wo_sb[:], wo_d[:])
            if b > 1:
                phase_a1(1)
            phase_d(0)
            for bi in range(1, b):
                phase_b(bi)
                phase_c(bi)
                phase_d(bi)

    nc.compile()
    return nc


# ----------------------------------------------------------------------------
# host-side constants and per-core input slicing
# ----------------------------------------------------------------------------

def _rope_tables(l):
    inv_freq = 1.0 / (10000.0 ** (np.arange(0, HEAD_DIM, 2, dtype=np.float32)
                                  / HEAD_DIM))                       # [32]
    t = np.arange(l, dtype=np.float32)
    fr = t[None, :] * inv_freq[:, None]                              # [32, l]
    cos1 = np.cos(np.concatenate([fr, fr], axis=0))                  # [64, l]
    sin1 = np.sin(np.concatenate([fr, fr], axis=0))                  # [64, l]
    sinm1 = np.concatenate([-sin1[:32], sin1[32:]], axis=0)          # sign-folded
    # half-swapped so the stt source base partition matches the operand rows
    sinswap1 = np.concatenate([sinm1[32:], sinm1[:32]], axis=0)
    cos = np.tile(cos1, (HPC, 1)).astype(np.float32)                 # [128, l]
    sinswap = np.tile(sinswap1, (HPC, 1)).astype(np.float32)
    return cos, sinswap


def _mask_strip():
    # strip[r, u] = 1 iff u >= r + 384; diagonal block at offset d uses
    # cols [384-d : 896-d] so that mask[r, c] = (c >= r + d)
    r = np.arange(128)[:, None]
    u = np.arange(896)[None, :]
    return (u >= r + 384).astype(np.float32)


def core_inputs(x_bf, ln_w, ln_b, W_in, W_out, c, l=L):
    """Build the per-core input map for core c (pure numpy).

    x_bf: [T, HIDDEN] bf16 (pre-cast once by the caller)."""
    import ml_dtypes
    bf16 = ml_dtypes.bfloat16
    ln_w = np.asarray(ln_w, np.float32)
    ln_b = np.asarray(ln_b, np.float32)
    W_in = np.asarray(W_in, np.float32)
    W_out = np.asarray(W_out, np.float32)

    qc = slice(c * QS, (c + 1) * QS)
    kc = slice(HIDDEN + c * QS, HIDDEN + (c + 1) * QS)
    vc = slice(2 * HIDDEN + c * QS, 2 * HIDDEN + (c + 1) * QS)
    fc = slice(3 * HIDDEN + c * FFS, 3 * HIDDEN + (c + 1) * FFS)
    w_raw = np.concatenate(
        [W_in[:, qc], W_in[:, kc], W_in[:, vc], W_in[:, fc]], axis=1)  # [1024, 896]
    w_slice = w_raw * ln_w[:, None]
    # device layout [128, KOUT, WSL]: w_arr[p, o, f] = w_slice[o*128+p, f]
    w_arr = np.ascontiguousarray(
        w_slice.reshape(KOUT, 128, WSL).transpose(1, 0, 2).astype(bf16))
    h_bias = (ln_b @ w_raw)                                            # [896]
    hb_arr = np.ascontiguousarray(
        h_bias.reshape(WSL // 128, 128).T.astype(np.float32))          # [128, 7]
    wo_slice = np.concatenate(
        [W_out[c * QS:(c + 1) * QS, :],
         W_out[HIDDEN + c * FFS: HIDDEN + (c + 1) * FFS, :]], axis=0)  # [640, 1024]
    wo_arr = np.ascontiguousarray(
        wo_slice.reshape(WOK, 128, HIDDEN).transpose(1, 0, 2).astype(bf16))

    cos, sinm = _rope_tables(l)
    return {
        "x": x_bf,
        "w_in": w_arr,
        "w_out": wo_arr,
        "h_bias": hb_arr,
        "cos_t": np.ascontiguousarray(cos.astype(bf16)),
        "sinm_t": np.ascontiguousarray(sinm.astype(bf16)),
        "mask_t": np.ascontiguousarray(_mask_strip().astype(bf16)),
    }


# ----------------------------------------------------------------------------
# entry point
# ----------------------------------------------------------------------------

_PROG_CACHE = {}


def kernel(x, ln_w, ln_b, W_in, W_out):
    global LAST_RESULTS
    import ml_dtypes
    from concourse import bass_utils
    from concourse.bass_interp import get_hw_module

    x = np.asarray(x, np.float32)
    b, l = x.shape[0], x.shape[1]

    key = (b, l)
    if key not in _PROG_CACHE:
        _PROG_CACHE[key] = build_program(b=b, l=l, debug=False)
    nc = _PROG_CACHE[key]

    x_bf = np.ascontiguousarray(
        x.reshape(b * l, HIDDEN).astype(ml_dtypes.bfloat16))
    in_maps = [core_inputs(x_bf, ln_w, ln_b, W_in, W_out, c, l=l)
               for c in range(NCORES)]

    old_m = nc.m
    nc.m = get_hw_module(nc.m)
    try:
        res = bass_utils.run_bass_kernel_spmd(
            nc, in_maps, core_ids=list(range(NCORES)),
            trace=bool(int(__import__("os").environ.get("BASS_TRACE_RUN", "0"))))
    finally:
        nc.m = old_m
    LAST_RESULTS = res

    acc = np.zeros((b * l, HIDDEN), np.float64)
    for r in res.results:
        acc += r["out"].astype(np.float64)
    return acc.reshape(b, l, HIDDEN).astype(np.float32)


# revision 73
# speedup vs baseline: 1.2077x; 1.0269x over previous
"""Trainium2 Bass kernel: PaLM-style parallel attention + FF transformer block.

Tensor-parallel over 8 NeuronCores: each core owns 2 heads (128 q/k/v cols of
W_in), 512 FF cols, and the matching 640 rows of W_out.  Each core computes a
full-shape partial output (bf16); the host sums the 8 partials (row-parallel
W_out).

v2: all matmuls in bf16 (2.4 GHz PE clock vs 1.2 GHz for fp32r), xbar
DMA-transposes instead of PE transposes, x fed as bf16 and read once per
batch, LN stats software-pipelined into the in-projection, attention
score->prob->PV chain software-pipelined so the PE never waits on the Act
engine, evictions spread across DVE/GpSimd, output written as bf16 partials.

Per-core dataflow (K-contractions on partitions):
  per 512-token chunk: x tiles (bf16) -> bn_stats -> mu/rs -> xn (bf16)
    -> xbar-transpose -> xnT [d, t]
  hT = W_slice^T @ xnT   (q,k rope-fused eviction; v re-transposed token-major
                          via xbar next to ones columns; ff -> gelu)
  per (i-chunk, head): ST[j,i] = kT^T q, PT = exp(0.125*ST) * causal_mask,
                       OT[0:64]=V^T PT accum, OT[64:128]=col-sums (ones cols)
  out_partial = [oT; ffgT]^T @ Wo_slice   (bf16 full-shape, host-summed)
"""

import numpy as np

HEADS = 16
HEAD_DIM = 64
HIDDEN = 1024
EXPF = 4
B = 2
L = 2048
NCORES = 8
HPC = HEADS // NCORES            # heads per core = 2
QS = HPC * HEAD_DIM              # per-core q/k/v width = 128
FFS = EXPF * HIDDEN // NCORES    # per-core ff width = 512
WSL = 3 * QS + FFS               # per-core W_in slice width = 896
KOUT = HIDDEN // 128             # 8 k-subtiles for hidden contraction
WOK = (QS + FFS) // 128          # 5 k-subtiles for out-proj contraction
LN_EPS = 1e-5

LAST_RESULTS = None  # BassKernelResults of the most recent HW run (for test.py)


# ----------------------------------------------------------------------------
# program builder
# ----------------------------------------------------------------------------

def build_program(b=B, l=L, debug=False, sim_gelu=False, opts=None,
                  dump=False):
    import concourse.bass as bass
    import concourse.mybir as mybir
    import concourse.tile as tile
    from concourse import bacc

    T = b * l
    NT = l // 128      # 128-token tiles per batch
    NC = l // 512      # 512-token chunks per batch
    f32 = mybir.dt.float32
    bf16 = mybir.dt.bfloat16
    AF = mybir.ActivationFunctionType
    OP = mybir.AluOpType

    opts = {**(opts or {})}
    nc = bacc.Bacc("TRN2", target_bir_lowering=False, debug=debug)

    x_d = nc.declare_dram_parameter("x", [T, HIDDEN], bf16, isOutput=False)
    w_d = nc.declare_dram_parameter("w_in", [128, KOUT, WSL], bf16, isOutput=False)
    wo_d = nc.declare_dram_parameter("w_out", [128, WOK, HIDDEN], bf16,
                                     isOutput=False)
    hb_d = nc.declare_dram_parameter("h_bias", [128, WSL // 128], f32,
                                     isOutput=False)
    cos_d = nc.declare_dram_parameter("cos_t", [128, l], bf16, isOutput=False)
    sinm_d = nc.declare_dram_parameter("sinm_t", [128, l], bf16, isOutput=False)
    mask_d = nc.declare_dram_parameter("mask_t", [128, 896], bf16, isOutput=False)
    out_d = nc.declare_dram_parameter("out", [T, HIDDEN], bf16, isOutput=True)
    if dump:
        dbg = {
            "d_xnT": nc.declare_dram_parameter(
                "d_xnT", [b, l // 512, 128, 4, KOUT, 128], bf16,
                isOutput=True),
            "d_qT": nc.declare_dram_parameter(
                "d_qT", [b, 128, l], bf16, isOutput=True),
            "d_kT": nc.declare_dram_parameter(
                "d_kT", [b, 128, l], bf16, isOutput=True),
            "d_vext": nc.declare_dram_parameter(
                "d_vext", [b, 128, l // 128, 256], bf16, isOutput=True),
            "d_ffgT": nc.declare_dram_parameter(
                "d_ffgT", [b, 128, EXPF, l], bf16, isOutput=True),
            "d_oT": nc.declare_dram_parameter(
                "d_oT", [b, 128, l], bf16, isOutput=True),
            "d_murs": nc.declare_dram_parameter(
                "d_murs", [2, 128, b * (l // 128)], f32, isOutput=True),
        }

    with tile.TileContext(nc) as tc:
        from contextlib import ExitStack
        with ExitStack() as ctx:
            const = ctx.enter_context(tc.tile_pool(name="const", bufs=1))
            strips = ctx.enter_context(tc.tile_pool(name="strips", bufs=1))
            xpool = ctx.enter_context(tc.tile_pool(name="xpool", bufs=12))
            stash = ctx.enter_context(tc.tile_pool(name="stash", bufs=1))
            stats = ctx.enter_context(tc.tile_pool(name="stats", bufs=4))
            xnpool = ctx.enter_context(tc.tile_pool(name="xnpool", bufs=4))
            xntp = ctx.enter_context(tc.tile_pool(name="xntp", bufs=2))
            work = ctx.enter_context(tc.tile_pool(name="work", bufs=2))
            vtpool = ctx.enter_context(tc.tile_pool(name="vtpool", bufs=2))
            ptp = ctx.enter_context(tc.tile_pool(name="ptp", bufs=3))
            sums_p = ctx.enter_context(tc.tile_pool(name="sums", bufs=2))
            obuf = ctx.enter_context(tc.tile_pool(name="obuf", bufs=2))
            psA = ctx.enter_context(tc.tile_pool(name="psA", bufs=4,
                                                 space="PSUM"))
            psS = ctx.enter_context(tc.tile_pool(name="psS", bufs=2,
                                                 space="PSUM"))

            # batch-0 chunk-0 x tiles first: their consumers gate the PE start
            x0_tiles = []
            for t4 in range(4):
                xt = xpool.tile([128, HIDDEN], bf16, tag="xt", name="xtb")
                nc.sync.dma_start(xt[:], x_d[t4 * 128:(t4 + 1) * 128, :])
                x0_tiles.append(xt)

            # constants (host pre-arranged; all contiguous single DMAs);
            # wo_sb and mask_sb loads are deferred past the startup window
            w_sb = const.tile([128, KOUT, WSL], bf16)
            nc.sync.dma_start(w_sb[:], w_d[:])
            cos_sb = const.tile([128, l], bf16)
            nc.sync.dma_start(cos_sb[:], cos_d[:])
            sinm_sb = const.tile([128, l], bf16)
            nc.sync.dma_start(sinm_sb[:], sinm_d[:])
            hb_sb = const.tile([128, WSL // 128], f32)
            nc.sync.dma_start(hb_sb[:], hb_d[:])
            wo_sb = const.tile([128, WOK, HIDDEN], bf16)
            mask_sb = const.tile([128, 896], bf16)
            eps_sb = const.tile([128, 1], f32)
            nc.vector.memset(eps_sb[:], LN_EPS)
            onesb_sb = const.tile([128, 1], bf16)
            nc.vector.memset(onesb_sb[:], 1.0)

            mu_all = const.tile([128, b * NT], f32, tag="mu_all")
            var_all = const.tile([128, b * NT], f32, tag="var_all")
            sd_all = const.tile([128, b * NT], f32, tag="sd_all")
            rs_all = const.tile([128, b * NT], f32, tag="rs_all")
            xstash_of = {}
            strips_of = {}

            def stats_tile(bi, tt, xt):
                # LN statistics for one 128-token tile (DVE only)
                gt = bi * NT + tt
                st6 = stats.tile([128, 2, 6], f32, tag="st6")
                nc.vector.bn_stats(st6[:, 0, :], xt[:, 0:512])
                nc.vector.bn_stats(st6[:, 1, :], xt[:, 512:1024])
                mv = stats.tile([128, 2], f32, tag="mv")
                nc.vector.bn_aggr(mv[:], st6[:])
                nc.vector.tensor_copy(mu_all[:, gt:gt + 1], mv[:, 0:1])
                nc.vector.tensor_copy(var_all[:, gt:gt + 1], mv[:, 1:2])

            def rs_cols(c0, n):
                # rs = 1/sqrt(var+eps) for stats columns [c0, c0+n)
                cs = slice(c0, c0 + n)
                nc.scalar.activation(sd_all[:, cs], var_all[:, cs],
                                     AF.Sqrt, bias=eps_sb[:])
                nc.vector.reciprocal_approx_fast(rs_all[:, cs],
                                                 sd_all[:, cs])

            def phase_b(bi):
                r0b = bi * l
                mu_s = mu_all[:, bi * NT:(bi + 1) * NT]
                rs_s = rs_all[:, bi * NT:(bi + 1) * NT]
                qT = strips.tile([128, l], bf16, tag="qT")
                kT = strips.tile([128, l], bf16, tag="kT")
                ffgT = strips.tile([128, EXPF, l], bf16, tag="ffgT")
                oT = strips.tile([128, l], bf16, tag="oT")
                vext = strips.tile([128, NT, 256], bf16, tag="vext")
                strips_of[bi] = (qT, kT, ffgT, oT, vext)

                # v_ext ones columns (produce the softmax row sums in PV)
                nc.vector.tensor_copy(
                    vext[:, :, 64:128],
                    onesb_sb[:, :, None].to_broadcast([128, NT, 64]))
                nc.vector.tensor_copy(
                    vext[:, :, 192:256],
                    onesb_sb[:, :, None].to_broadcast([128, NT, 64]))

                xt_of = {}

                def load_chunk(cj):
                    if bi != 0 or cj >= NC or cj in xt_of:
                        return
                    if cj == 0:
                        xt_of[0] = x0_tiles
                        return
                    tiles = []
                    for t4 in range(4):
                        tt = cj * 4 + t4
                        xt = xpool.tile([128, HIDDEN], bf16, tag="xt",
                                        name="xtb")
                        nc.sync.dma_start(
                            xt[:], x_d[r0b + tt * 128: r0b + (tt + 1) * 128, :])
                        tiles.append(xt)
                    xt_of[cj] = tiles

                def stats_chunk(cj):
                    if bi != 0 or cj >= NC:
                        return
                    for t4 in range(4):
                        stats_tile(bi, cj * 4 + t4, xt_of[cj][t4])
                    rs_cols(bi * NT + cj * 4, 4)

                if bi == 0:
                    load_chunk(0)
                    stats_chunk(0)

                for ci in range(NC):
                    if bi == 0:
                        xts = xt_of.pop(ci)
                    else:
                        xst = xstash_of[bi]
                        xts = [xst[:, ci * 4 + t4, :] for t4 in range(4)]
                    # xnT4[d, t4, ko, t] = xn4[t, t4, ko*128+d] via one (or
                    # four, for the latency-critical first chunk) xbar DMAs
                    xnT4 = xntp.tile([128, 4, KOUT, 128], bf16, tag="xnT")
                    xn4 = xnpool.tile([128, 4, HIDDEN], bf16, tag="xn")
                    for t4 in range(4):
                        tt = ci * 4 + t4
                        nc.vector.tensor_scalar(
                            out=xn4[:, t4, :], in0=xts[t4][:],
                            scalar1=mu_s[:, tt:tt + 1],
                            scalar2=rs_s[:, tt:tt + 1],
                            op0=OP.subtract, op1=OP.mult)
                        if bi == 0 and ci == 0:
                            nc.sync.dma_start(
                                xnT4[:, t4, :, :], xn4[:, t4, :],
                                transpose=True)
                    if not (bi == 0 and ci == 0):
                        nc.sync.dma_start(
                            xnT4[:].rearrange("p a b t -> p (a b) t"),
                            xn4[:].rearrange("p a d -> p (a d)"),
                            transpose=True)
                    if dump:
                        nc.sync.dma_start(dbg["d_xnT"][bi, ci], xnT4[:])
                    for m in range(3 + EXPF):
                        hps = psA.tile([128, 512], f32, tag="p512", name="hps")
                        for ko in range(KOUT):
                            nc.tensor.matmul(
                                hps[:],
                                w_sb[:, ko, m * 128:(m + 1) * 128],
                                xnT4[:, :, ko, :],
                                start=(ko == 0), stop=(ko == KOUT - 1))
                        if m < 2:
                            # q/k: rope-fused eviction (+ h bias)
                            dst = (qT if m == 0 else kT)[:, ci * 512:(ci + 1) * 512]
                            cs = cos_sb[:, ci * 512:(ci + 1) * 512]
                            sn = sinm_sb[:, ci * 512:(ci + 1) * 512]
                            tmpc = work.tile([128, 512], f32, tag="rtmp1")
                            t2 = work.tile([128, 512], f32, tag="rtmp2")
                            nc.vector.scalar_tensor_tensor(
                                tmpc[:], hps[:], hb_sb[:, m:m + 1], cs,
                                OP.add, OP.mult)
                            for h in range(HPC):
                                r0 = h * 64
                                nc.vector.scalar_tensor_tensor(
                                    t2[r0:r0 + 32, :], hps[r0 + 32:r0 + 64, :],
                                    hb_sb[r0 + 32:r0 + 64, m:m + 1],
                                    sn[r0 + 32:r0 + 64, :], OP.add, OP.mult)
                                nc.vector.scalar_tensor_tensor(
                                    t2[r0 + 32:r0 + 64, :], hps[r0:r0 + 32, :],
                                    hb_sb[r0:r0 + 32, m:m + 1],
                                    sn[r0:r0 + 32, :], OP.add, OP.mult)
                            nc.vector.tensor_add(dst, tmpc[:], t2[:])
                        elif m == 2:
                            # v: bias, then xbar re-transpose to token-major
                            vtmp = vtpool.tile([128, 512], bf16, tag="vtmp")
                            nc.vector.tensor_scalar_add(
                                vtmp[:], hps[:], hb_sb[:, 2:3])
                            # vext[t, ci*4+o, 0:64]    = v_h0[t]
                            # vext[t, ci*4+o, 128:192] = v_h1[t]
                            nc.sync.dma_start_transpose(
                                vext[:, ci * 4:(ci + 1) * 4, 0:64],
                                vtmp[0:64, :])
                            nc.sync.dma_start_transpose(
                                vext[:, ci * 4:(ci + 1) * 4, 128:192],
                                vtmp[64:128, :])
                            # lookahead loads + stats slot into the DVE queue
                            # here, under the gelu-only eviction tail
                            load_chunk(ci + 1)
                            load_chunk(ci + 2)
                            stats_chunk(ci + 1)
                        else:
                            nc.scalar.activation(
                                ffgT[:, m - 3, ci * 512:(ci + 1) * 512], hps[:],
                                AF.Identity if sim_gelu else AF.Gelu,
                                bias=hb_sb[:, m:m + 1])
                if dump:
                    nc.sync.dma_start(dbg["d_qT"][bi], qT[:])
                    nc.sync.dma_start(dbg["d_kT"][bi], kT[:])
                    nc.sync.dma_start(dbg["d_vext"][bi], vext[:])
                    nc.sync.dma_start(dbg["d_ffgT"][bi], ffgT[:])
                    nc.sync.dma_start(dbg["d_murs"][0], mu_all[:])
                    nc.sync.dma_start(dbg["d_murs"][1], rs_all[:])

            def phase_c(bi):
                qT, kT, ffgT, oT, vext = strips_of[bi]
                for ic in range(NC):
                    ot = [psA.tile([128, 512], f32, tag="p512",
                                   name=f"ot{_h}")
                          for _h in range(HPC)]
                    njt = (ic + 1) * 4
                    pts = {}
                    for step in range(njt + 1):
                        if step < njt:
                            jt = step
                            st2 = psS.tile([128, 1024], f32, tag="st2")
                            for h in range(HPC):
                                nc.tensor.matmul(
                                    st2[:, h * 512:(h + 1) * 512],
                                    kT[h * 64:(h + 1) * 64,
                                       jt * 128:(jt + 1) * 128],
                                    qT[h * 64:(h + 1) * 64,
                                       ic * 512:(ic + 1) * 512],
                                    start=True, stop=True)
                            pt2 = ptp.tile([128, 1024], bf16, tag="pt")
                            nc.scalar.activation(
                                pt2[:], st2[:], AF.Exp,
                                scale=float(HEAD_DIM) ** -0.5)
                            d = jt * 128 - ic * 512
                            if d >= 0:
                                nc.vector.tensor_tensor(
                                    pt2[:].rearrange("p (g c) -> p g c", c=512),
                                    pt2[:].rearrange("p (g c) -> p g c", c=512),
                                    mask_sb[:, None, 384 - d:896 - d]
                                    .to_broadcast([128, HPC, 512]),
                                    OP.mult)
                            pts[jt] = pt2
                        if step >= 1:
                            jt = step - 1
                            pt2 = pts.pop(jt)
                            for h in range(HPC):
                                nc.tensor.matmul(
                                    ot[h][:],
                                    vext[:, jt, h * 128:(h + 1) * 128],
                                    pt2[:, h * 512:(h + 1) * 512],
                                    start=(jt == 0), stop=(jt == njt - 1))
                    for h in range(HPC):
                        sums_sb = sums_p.tile([64, 512], f32, tag="sums")
                        # approx recip needs an SBUF source (PSUM reads are
                        # silently wrong on HW for custom DVE ops)
                        nc.vector.tensor_copy(sums_sb[:], ot[h][64:128, :])
                        nc.vector.reciprocal_approx_fast(
                            sums_sb[:], sums_sb[:])
                        nc.vector.tensor_mul(
                            oT[h * 64:(h + 1) * 64, ic * 512:(ic + 1) * 512],
                            ot[h][0:64, :], sums_sb[:])
                if dump:
                    nc.sync.dma_start(dbg["d_oT"][bi], oT[:])

            def a1_load(bi):
                # batch-bi x stash (one big DMA; transfer hides under C0)
                r0b = bi * l
                xst = stash.tile([128, NT, HIDDEN], bf16, tag="xst",
                                 name=f"xst{bi}")
                xstash_of[bi] = xst
                nc.sync.dma_start(
                    xst[:],
                    x_d[r0b:r0b + l, :].rearrange("(o p) d -> p o d", p=128))

            def phase_d(bi, stats_for=None):
                # stats_for: batch whose stash LN stats are woven in one per
                # tt (keeps them late in the DVE queue so the scheduler can't
                # hoist them ahead of phase-B-tail work they'd block)
                r0b = bi * l
                qT, kT, ffgT, oT, vext = strips_of[bi]
                for tt in range(NT):
                    if stats_for is not None:
                        stats_tile(stats_for, tt, xstash_of[stats_for][:, tt, :])
                        if tt == NT - 1:
                            rs_cols(stats_for * NT, NT)
                    ob = obuf.tile([128, HIDDEN], bf16, tag="ob")
                    for n2 in range(2):
                        ops = psA.tile([128, 512], f32, tag="p512", name="ops")
                        nc.tensor.matmul(
                            ops[:], oT[:, tt * 128:(tt + 1) * 128],
                            wo_sb[:, 0, n2 * 512:(n2 + 1) * 512],
                            start=True, stop=False)
                        for kk in range(EXPF):
                            nc.tensor.matmul(
                                ops[:], ffgT[:, kk, tt * 128:(tt + 1) * 128],
                                wo_sb[:, kk + 1, n2 * 512:(n2 + 1) * 512],
                                start=False, stop=(kk == EXPF - 1))
                        nc.scalar.activation(
                            ob[:, n2 * 512:(n2 + 1) * 512], ops[:], AF.Copy)
                    nc.sync.dma_start(
                        out_d[r0b + tt * 128: r0b + (tt + 1) * 128, :], ob[:])

            phase_b(0)
            nc.sync.dma_start(mask_sb[:], mask_d[:])
            if b > 1:
                a1_load(1)
            phase_c(0)
            nc.sync.dma_start(wo_sb[:], wo_d[:])
            phase_d(0, stats_for=1 if b > 1 else None)
            for bi in range(1, b):
                phase_b(bi)
                phase_c(bi)
                phase_d(bi)

    nc.compile()
    return nc


# ----------------------------------------------------------------------------
# host-side constants and per-core input slicing
# ----------------------------------------------------------------------------

def _rope_tables(l):
    inv_freq = 1.0 / (10000.0 ** (np.arange(0, HEAD_DIM, 2, dtype=np.float32)
                                  / HEAD_DIM))                       # [32]
    t = np.arange(l, dtype=np.float32)
    fr = t[None, :] * inv_freq[:, None]                              # [32, l]
    cos1 = np.cos(np.concatenate([fr, fr], axis=0))                  # [64, l]
    sin1 = np.sin(np.concatenate([fr, fr], axis=0))                  # [64, l]
    sinm1 = np.concatenate([-sin1[:32], sin1[32:]], axis=0)          # sign-folded
    # half-swapped so the stt source base partition matches the operand rows
    sinswap1 = np.concatenate([sinm1[32:], sinm1[:32]], axis=0)
    cos = np.tile(cos1, (HPC, 1)).astype(np.float32)                 # [128, l]
    sinswap = np.tile(sinswap1, (HPC, 1)).astype(np.float32)
    return cos, sinswap


def _mask_strip():
    # strip[r, u] = 1 iff u >= r + 384; diagonal block at offset d uses
    # cols [384-d : 896-d] so that mask[r, c] = (c >= r + d)
    r = np.arange(128)[:, None]
    u = np.arange(896)[None, :]
    return (u >= r + 384).astype(np.float32)


def core_inputs(x_bf, ln_w, ln_b, W_in, W_out, c, l=L):
    """Build the per-core input map for core c (pure numpy).

    x_bf: [T, HIDDEN] bf16 (pre-cast once by the caller)."""
    import ml_dtypes
    bf16 = ml_dtypes.bfloat16
    ln_w = np.asarray(ln_w, np.float32)
    ln_b = np.asarray(ln_b, np.float32)
    W_in = np.asarray(W_in, np.float32)
    W_out = np.asarray(W_out, np.float32)

    qc = slice(c * QS, (c + 1) * QS)
    kc = slice(HIDDEN + c * QS, HIDDEN + (c + 1) * QS)
    vc = slice(2 * HIDDEN + c * QS, 2 * HIDDEN + (c + 1) * QS)
    fc = slice(3 * HIDDEN + c * FFS, 3 * HIDDEN + (c + 1) * FFS)
    w_raw = np.concatenate(
        [W_in[:, qc], W_in[:, kc], W_in[:, vc], W_in[:, fc]], axis=1)  # [1024, 896]
    w_slice = w_raw * ln_w[:, None]
    # device layout [128, KOUT, WSL]: w_arr[p, o, f] = w_slice[o*128+p, f]
    w_arr = np.ascontiguousarray(
        w_slice.reshape(KOUT, 128, WSL).transpose(1, 0, 2).astype(bf16))
    h_bias = (ln_b @ w_raw)                                            # [896]
    hb_arr = np.ascontiguousarray(
        h_bias.reshape(WSL // 128, 128).T.astype(np.float32))          # [128, 7]
    wo_slice = np.concatenate(
        [W_out[c * QS:(c + 1) * QS, :],
         W_out[HIDDEN + c * FFS: HIDDEN + (c + 1) * FFS, :]], axis=0)  # [640, 1024]
    wo_arr = np.ascontiguousarray(
        wo_slice.reshape(WOK, 128, HIDDEN).transpose(1, 0, 2).astype(bf16))

    cos, sinm = _rope_tables(l)
    return {
        "x": x_bf,
        "w_in": w_arr,
        "w_out": wo_arr,
        "h_bias": hb_arr,
        "cos_t": np.ascontiguousarray(cos.astype(bf16)),
        "sinm_t": np.ascontiguousarray(sinm.astype(bf16)),
        "mask_t": np.ascontiguousarray(_mask_strip().astype(bf16)),
    }


# ----------------------------------------------------------------------------
# entry point
# ----------------------------------------------------------------------------

_PROG_CACHE = {}


def kernel(x, ln_w, ln_b, W_in, W_out):
    global LAST_RESULTS
    import ml_dtypes
    from concourse import bass_utils
    from concourse.bass_interp import get_hw_module

    x = np.asarray(x, np.float32)
    b, l = x.shape[0], x.shape[1]

    key = (b, l)
    if key not in _PROG_CACHE:
        _PROG_CACHE[key] = build_program(b=b, l=l, debug=False)
    nc = _PROG_CACHE[key]

    x_bf = np.ascontiguousarray(
        x.reshape(b * l, HIDDEN).astype(ml_dtypes.bfloat16))
    in_maps = [core_inputs(x_bf, ln_w, ln_b, W_in, W_out, c, l=l)
               for c in range(NCORES)]

    old_m = nc.m
    nc.m = get_hw_module(nc.m)
    try:
        res = bass_utils.run_bass_kernel_spmd(
            nc, in_maps, core_ids=list(range(NCORES)),
            trace=bool(int(__import__("os").environ.get("BASS_TRACE_RUN", "0"))))
    finally:
        nc.m = old_m
    LAST_RESULTS = res

    acc = np.zeros((b * l, HIDDEN), np.float64)
    for r in res.results:
        acc += r["out"].astype(np.float64)
    return acc.reshape(b, l, HIDDEN).astype(np.float32)


# revision 78
# speedup vs baseline: 1.2388x; 1.0257x over previous
"""Trainium2 Bass kernel: PaLM-style parallel attention + FF transformer block.

Tensor-parallel over 8 NeuronCores: each core owns 2 heads (128 q/k/v cols of
W_in), 512 FF cols, and the matching 640 rows of W_out.  Each core computes a
full-shape partial output (bf16); the host sums the 8 partials (row-parallel
W_out).

v2: all matmuls in bf16 (2.4 GHz PE clock vs 1.2 GHz for fp32r), xbar
DMA-transposes instead of PE transposes, x fed as bf16 and read once per
batch, LN stats software-pipelined into the in-projection, attention
score->prob->PV chain software-pipelined so the PE never waits on the Act
engine, evictions spread across DVE/GpSimd, output written as bf16 partials.

Per-core dataflow (K-contractions on partitions):
  per 512-token chunk: x tiles (bf16) -> bn_stats -> mu/rs -> xn (bf16)
    -> xbar-transpose -> xnT [d, t]
  hT = W_slice^T @ xnT   (q,k rope-fused eviction; v re-transposed token-major
                          via xbar next to ones columns; ff -> gelu)
  per (i-chunk, head): ST[j,i] = kT^T q, PT = exp(0.125*ST) * causal_mask,
                       OT[0:64]=V^T PT accum, OT[64:128]=col-sums (ones cols)
  out_partial = [oT; ffgT]^T @ Wo_slice   (bf16 full-shape, host-summed)
"""

import numpy as np

HEADS = 16
HEAD_DIM = 64
HIDDEN = 1024
EXPF = 4
B = 2
L = 2048
NCORES = 8
HPC = HEADS // NCORES            # heads per core = 2
QS = HPC * HEAD_DIM              # per-core q/k/v width = 128
FFS = EXPF * HIDDEN // NCORES    # per-core ff width = 512
WSL = 3 * QS + FFS               # per-core W_in slice width = 896
KOUT = HIDDEN // 128             # 8 k-subtiles for hidden contraction
WOK = (QS + FFS) // 128          # 5 k-subtiles for out-proj contraction
LN_EPS = 1e-5

LAST_RESULTS = None  # BassKernelResults of the most recent HW run (for test.py)


# ----------------------------------------------------------------------------
# program builder
# ----------------------------------------------------------------------------

def build_program(b=B, l=L, debug=False, sim_gelu=False, opts=None,
                  dump=False):
    import concourse.bass as bass
    import concourse.mybir as mybir
    import concourse.tile as tile
    from concourse import bacc

    T = b * l
    NT = l // 128      # 128-token tiles per batch
    NC = l // 512      # 512-token chunks per batch
    f32 = mybir.dt.float32
    bf16 = mybir.dt.bfloat16
    AF = mybir.ActivationFunctionType
    OP = mybir.AluOpType

    opts = {**(opts or {})}
    nc = bacc.Bacc("TRN2", target_bir_lowering=False, debug=debug)

    x_d = nc.declare_dram_parameter("x", [T, HIDDEN], bf16, isOutput=False)
    w_d = nc.declare_dram_parameter("w_in", [128, KOUT, WSL], bf16, isOutput=False)
    wo_d = nc.declare_dram_parameter("w_out", [128, WOK, HIDDEN], bf16,
                                     isOutput=False)
    hb_d = nc.declare_dram_parameter("h_bias", [128, WSL // 128], f32,
                                     isOutput=False)
    cos_d = nc.declare_dram_parameter("cos_t", [128, l], bf16, isOutput=False)
    sinm_d = nc.declare_dram_parameter("sinm_t", [128, l], bf16, isOutput=False)
    mask_d = nc.declare_dram_parameter("mask_t", [128, 896], bf16, isOutput=False)
    out_d = nc.declare_dram_parameter("out", [T, HIDDEN], bf16, isOutput=True)
    if dump:
        dbg = {
            "d_xnT": nc.declare_dram_parameter(
                "d_xnT", [b, l // 512, 128, 4, KOUT, 128], bf16,
                isOutput=True),
            "d_qT": nc.declare_dram_parameter(
                "d_qT", [b, 128, l], bf16, isOutput=True),
            "d_kT": nc.declare_dram_parameter(
                "d_kT", [b, 128, l], bf16, isOutput=True),
            "d_vext": nc.declare_dram_parameter(
                "d_vext", [b, 128, l // 128, 256], bf16, isOutput=True),
            "d_ffgT": nc.declare_dram_parameter(
                "d_ffgT", [b, 128, EXPF, l], bf16, isOutput=True),
            "d_oT": nc.declare_dram_parameter(
                "d_oT", [b, 128, l], bf16, isOutput=True),
            "d_murs": nc.declare_dram_parameter(
                "d_murs", [2, 128, b * (l // 128)], f32, isOutput=True),
        }

    with tile.TileContext(nc) as tc:
        from contextlib import ExitStack
        with ExitStack() as ctx:
            const = ctx.enter_context(tc.tile_pool(name="const", bufs=1))
            strips = ctx.enter_context(tc.tile_pool(name="strips", bufs=1))
            xpool = ctx.enter_context(tc.tile_pool(name="xpool", bufs=10))
            stash = ctx.enter_context(tc.tile_pool(name="stash", bufs=1))
            stats = ctx.enter_context(tc.tile_pool(name="stats", bufs=4))
            xnpool = ctx.enter_context(tc.tile_pool(name="xnpool", bufs=4))
            xntp = ctx.enter_context(tc.tile_pool(name="xntp", bufs=2))
            work = ctx.enter_context(tc.tile_pool(name="work", bufs=6))
            vtpool = ctx.enter_context(tc.tile_pool(name="vtpool", bufs=2))
            ptp = ctx.enter_context(tc.tile_pool(name="ptp", bufs=3))
            sums_p = ctx.enter_context(tc.tile_pool(name="sums", bufs=2))
            obuf = ctx.enter_context(tc.tile_pool(name="obuf", bufs=2))
            psA = ctx.enter_context(tc.tile_pool(name="psA", bufs=4,
                                                 space="PSUM"))
            psS = ctx.enter_context(tc.tile_pool(name="psS", bufs=2,
                                                 space="PSUM"))

            # batch-0 chunk-0 x tiles first: their consumers gate the PE start
            x0_tiles = []
            for t4 in range(4):
                xt = xpool.tile([128, HIDDEN], bf16, tag="xt", name="xtb")
                nc.sync.dma_start(xt[:], x_d[t4 * 128:(t4 + 1) * 128, :])
                x0_tiles.append(xt)

            # constants (host pre-arranged; all contiguous single DMAs);
            # wo_sb and mask_sb loads are deferred past the startup window
            w_sb = const.tile([128, KOUT, WSL], bf16)
            nc.sync.dma_start(w_sb[:], w_d[:])
            cos_sb = const.tile([128, l], bf16)
            nc.sync.dma_start(cos_sb[:], cos_d[:])
            sinm_sb = const.tile([128, l], bf16)
            nc.sync.dma_start(sinm_sb[:], sinm_d[:])
            hb_sb = const.tile([128, WSL // 128], f32)
            nc.sync.dma_start(hb_sb[:], hb_d[:])
            wo_sb = const.tile([128, WOK, HIDDEN], bf16)
            mask_sb = const.tile([128, 896], bf16)
            eps_sb = const.tile([128, 1], f32)
            nc.vector.memset(eps_sb[:], LN_EPS)
            onesb_sb = const.tile([128, 1], bf16)
            nc.vector.memset(onesb_sb[:], 1.0)

            mu_all = const.tile([128, b * NT], f32, tag="mu_all")
            var_all = const.tile([128, b * NT], f32, tag="var_all")
            sd_all = const.tile([128, b * NT], f32, tag="sd_all")
            rs_all = const.tile([128, b * NT], f32, tag="rs_all")
            xstash_of = {}
            strips_of = {}

            def stats_tile(bi, tt, xt):
                # LN statistics for one 128-token tile (DVE only)
                gt = bi * NT + tt
                st6 = stats.tile([128, 2, 6], f32, tag="st6")
                nc.vector.bn_stats(st6[:, 0, :], xt[:, 0:512])
                nc.vector.bn_stats(st6[:, 1, :], xt[:, 512:1024])
                mv = stats.tile([128, 2], f32, tag="mv")
                nc.vector.bn_aggr(mv[:], st6[:])
                nc.vector.tensor_copy(mu_all[:, gt:gt + 1], mv[:, 0:1])
                nc.vector.tensor_copy(var_all[:, gt:gt + 1], mv[:, 1:2])

            def rs_cols(c0, n):
                # rs = 1/sqrt(var+eps) for stats columns [c0, c0+n)
                cs = slice(c0, c0 + n)
                nc.scalar.activation(sd_all[:, cs], var_all[:, cs],
                                     AF.Sqrt, bias=eps_sb[:])
                nc.vector.reciprocal_approx_fast(rs_all[:, cs],
                                                 sd_all[:, cs])

            def phase_b(bi):
                r0b = bi * l
                mu_s = mu_all[:, bi * NT:(bi + 1) * NT]
                rs_s = rs_all[:, bi * NT:(bi + 1) * NT]
                qT = strips.tile([128, l], bf16, tag="qT")
                kT = strips.tile([128, l], bf16, tag="kT")
                ffgT = strips.tile([128, EXPF, l], bf16, tag="ffgT")
                oT = strips.tile([128, l], bf16, tag="oT")
                vext = strips.tile([128, NT, 256], bf16, tag="vext")
                strips_of[bi] = (qT, kT, ffgT, oT, vext)

                # v_ext ones columns (produce the softmax row sums in PV)
                nc.vector.tensor_copy(
                    vext[:, :, 64:128],
                    onesb_sb[:, :, None].to_broadcast([128, NT, 64]))
                nc.vector.tensor_copy(
                    vext[:, :, 192:256],
                    onesb_sb[:, :, None].to_broadcast([128, NT, 64]))

                xt_of = {}

                def load_chunk(cj):
                    if bi != 0 or cj >= NC or cj in xt_of:
                        return
                    if cj == 0:
                        xt_of[0] = x0_tiles
                        return
                    tiles = []
                    for t4 in range(4):
                        tt = cj * 4 + t4
                        xt = xpool.tile([128, HIDDEN], bf16, tag="xt",
                                        name="xtb")
                        nc.sync.dma_start(
                            xt[:], x_d[r0b + tt * 128: r0b + (tt + 1) * 128, :])
                        tiles.append(xt)
                    xt_of[cj] = tiles

                def stats_chunk(cj):
                    if bi != 0 or cj >= NC:
                        return
                    for t4 in range(4):
                        stats_tile(bi, cj * 4 + t4, xt_of[cj][t4])
                    rs_cols(bi * NT + cj * 4, 4)

                if bi == 0:
                    load_chunk(0)
                    stats_chunk(0)

                for ci in range(NC):
                    if bi == 0:
                        xts = xt_of.pop(ci)
                    else:
                        xst = xstash_of[bi]
                        xts = [xst[:, ci * 4 + t4, :] for t4 in range(4)]
                    # xnT4[d, t4, ko, t] = xn4[t, t4, ko*128+d] via one (or
                    # four, for the latency-critical first chunk) xbar DMAs
                    xnT4 = xntp.tile([128, 4, KOUT, 128], bf16, tag="xnT")
                    xn4 = xnpool.tile([128, 4, HIDDEN], bf16, tag="xn")
                    for t4 in range(4):
                        tt = ci * 4 + t4
                        nc.vector.tensor_scalar(
                            out=xn4[:, t4, :], in0=xts[t4][:],
                            scalar1=mu_s[:, tt:tt + 1],
                            scalar2=rs_s[:, tt:tt + 1],
                            op0=OP.subtract, op1=OP.mult)
                        if bi == 0 and ci == 0:
                            nc.sync.dma_start(
                                xnT4[:, t4, :, :], xn4[:, t4, :],
                                transpose=True)
                    if not (bi == 0 and ci == 0):
                        nc.sync.dma_start(
                            xnT4[:].rearrange("p a b t -> p (a b) t"),
                            xn4[:].rearrange("p a d -> p (a d)"),
                            transpose=True)
                    if dump:
                        nc.sync.dma_start(dbg["d_xnT"][bi, ci], xnT4[:])
                    for m in range(3 + EXPF):
                        hps = psA.tile([128, 512], f32, tag="p512", name="hps")
                        for ko in range(KOUT):
                            nc.tensor.matmul(
                                hps[:],
                                w_sb[:, ko, m * 128:(m + 1) * 128],
                                xnT4[:, :, ko, :],
                                start=(ko == 0), stop=(ko == KOUT - 1))
                        if m < 2:
                            # q/k: rope-fused eviction.  Act applies the h
                            # bias evicting PSUM->SBUF bf16 (COPY is in every
                            # act table); rope then runs as cheap all-bf16
                            # SBUF ops on the DVE.
                            dst = (qT if m == 0 else kT)[:, ci * 512:(ci + 1) * 512]
                            cs = cos_sb[:, ci * 512:(ci + 1) * 512]
                            sn = sinm_sb[:, ci * 512:(ci + 1) * 512]
                            hq = work.tile([128, 512], bf16, tag="hq")
                            nc.scalar.activation(hq[:], hps[:], AF.Identity,
                                                 bias=hb_sb[:, m:m + 1])
                            tmpc = work.tile([128, 512], bf16, tag="rtmp1")
                            t2 = work.tile([128, 512], bf16, tag="rtmp2")
                            nc.vector.tensor_mul(tmpc[:], hq[:], cs)
                            for h in range(HPC):
                                r0 = h * 64
                                nc.vector.tensor_mul(
                                    t2[r0:r0 + 32, :], hq[r0 + 32:r0 + 64, :],
                                    sn[r0 + 32:r0 + 64, :])
                                nc.vector.tensor_mul(
                                    t2[r0 + 32:r0 + 64, :], hq[r0:r0 + 32, :],
                                    sn[r0:r0 + 32, :])
                            nc.vector.tensor_add(dst, tmpc[:], t2[:])
                        elif m == 2:
                            # v: bias, then xbar re-transpose to token-major
                            vtmp = vtpool.tile([128, 512], bf16, tag="vtmp")
                            nc.vector.tensor_scalar_add(
                                vtmp[:], hps[:], hb_sb[:, 2:3])
                            # vext[t, ci*4+o, 0:64]    = v_h0[t]
                            # vext[t, ci*4+o, 128:192] = v_h1[t]
                            nc.sync.dma_start_transpose(
                                vext[:, ci * 4:(ci + 1) * 4, 0:64],
                                vtmp[0:64, :])
                            nc.sync.dma_start_transpose(
                                vext[:, ci * 4:(ci + 1) * 4, 128:192],
                                vtmp[64:128, :])
                            # lookahead loads + stats slot into the DVE queue
                            # here, under the gelu-only eviction tail
                            load_chunk(ci + 1)
                            load_chunk(ci + 2)
                            stats_chunk(ci + 1)
                        else:
                            nc.scalar.activation(
                                ffgT[:, m - 3, ci * 512:(ci + 1) * 512], hps[:],
                                AF.Identity if sim_gelu else AF.Gelu,
                                bias=hb_sb[:, m:m + 1])
                if dump:
                    nc.sync.dma_start(dbg["d_qT"][bi], qT[:])
                    nc.sync.dma_start(dbg["d_kT"][bi], kT[:])
                    nc.sync.dma_start(dbg["d_vext"][bi], vext[:])
                    nc.sync.dma_start(dbg["d_ffgT"][bi], ffgT[:])
                    nc.sync.dma_start(dbg["d_murs"][0], mu_all[:])
                    nc.sync.dma_start(dbg["d_murs"][1], rs_all[:])

            def phase_c(bi):
                qT, kT, ffgT, oT, vext = strips_of[bi]
                for ic in range(NC):
                    ot = [psA.tile([128, 512], f32, tag="p512",
                                   name=f"ot{_h}")
                          for _h in range(HPC)]
                    njt = (ic + 1) * 4
                    pts = {}
                    for step in range(njt + 1):
                        if step < njt:
                            jt = step
                            st2 = psS.tile([128, 1024], f32, tag="st2")
                            for h in range(HPC):
                                nc.tensor.matmul(
                                    st2[:, h * 512:(h + 1) * 512],
                                    kT[h * 64:(h + 1) * 64,
                                       jt * 128:(jt + 1) * 128],
                                    qT[h * 64:(h + 1) * 64,
                                       ic * 512:(ic + 1) * 512],
                                    start=True, stop=True)
                            pt2 = ptp.tile([128, 1024], bf16, tag="pt")
                            nc.scalar.activation(
                                pt2[:], st2[:], AF.Exp,
                                scale=float(HEAD_DIM) ** -0.5)
                            d = jt * 128 - ic * 512
                            if d >= 0:
                                nc.vector.tensor_tensor(
                                    pt2[:].rearrange("p (g c) -> p g c", c=512),
                                    pt2[:].rearrange("p (g c) -> p g c", c=512),
                                    mask_sb[:, None, 384 - d:896 - d]
                                    .to_broadcast([128, HPC, 512]),
                                    OP.mult)
                            pts[jt] = pt2
                        if step >= 1:
                            jt = step - 1
                            pt2 = pts.pop(jt)
                            for h in range(HPC):
                                nc.tensor.matmul(
                                    ot[h][:],
                                    vext[:, jt, h * 128:(h + 1) * 128],
                                    pt2[:, h * 512:(h + 1) * 512],
                                    start=(jt == 0), stop=(jt == njt - 1))
                    for h in range(HPC):
                        sums_sb = sums_p.tile([64, 512], f32, tag="sums")
                        # approx recip needs an SBUF source (PSUM reads are
                        # silently wrong on HW for custom DVE ops)
                        nc.vector.tensor_copy(sums_sb[:], ot[h][64:128, :])
                        nc.vector.reciprocal_approx_fast(
                            sums_sb[:], sums_sb[:])
                        nc.vector.tensor_mul(
                            oT[h * 64:(h + 1) * 64, ic * 512:(ic + 1) * 512],
                            ot[h][0:64, :], sums_sb[:])
                if dump:
                    nc.sync.dma_start(dbg["d_oT"][bi], oT[:])

            def a1_load(bi):
                # batch-bi x stash (one big DMA; transfer hides under C0)
                r0b = bi * l
                xst = stash.tile([128, NT, HIDDEN], bf16, tag="xst",
                                 name=f"xst{bi}")
                xstash_of[bi] = xst
                nc.sync.dma_start(
                    xst[:],
                    x_d[r0b:r0b + l, :].rearrange("(o p) d -> p o d", p=128))

            def phase_d(bi, stats_for=None):
                # stats_for: batch whose stash LN stats are woven in one per
                # tt (keeps them late in the DVE queue so the scheduler can't
                # hoist them ahead of phase-B-tail work they'd block)
                r0b = bi * l
                qT, kT, ffgT, oT, vext = strips_of[bi]
                for tt in range(NT):
                    if stats_for is not None:
                        stats_tile(stats_for, tt, xstash_of[stats_for][:, tt, :])
                        if tt == NT - 1:
                            rs_cols(stats_for * NT, NT)
                    ob = obuf.tile([128, HIDDEN], bf16, tag="ob")
                    for n2 in range(2):
                        ops = psA.tile([128, 512], f32, tag="p512", name="ops")
                        nc.tensor.matmul(
                            ops[:], oT[:, tt * 128:(tt + 1) * 128],
                            wo_sb[:, 0, n2 * 512:(n2 + 1) * 512],
                            start=True, stop=False)
                        for kk in range(EXPF):
                            nc.tensor.matmul(
                                ops[:], ffgT[:, kk, tt * 128:(tt + 1) * 128],
                                wo_sb[:, kk + 1, n2 * 512:(n2 + 1) * 512],
                                start=False, stop=(kk == EXPF - 1))
                        nc.scalar.activation(
                            ob[:, n2 * 512:(n2 + 1) * 512], ops[:], AF.Copy)
                    nc.sync.dma_start(
                        out_d[r0b + tt * 128: r0b + (tt + 1) * 128, :], ob[:])

            phase_b(0)
            nc.sync.dma_start(mask_sb[:], mask_d[:])
            if b > 1:
                a1_load(1)
            phase_c(0)
            nc.sync.dma_start(wo_sb[:], wo_d[:])
            phase_d(0, stats_for=1 if b > 1 else None)
            for bi in range(1, b):
                phase_b(bi)
                phase_c(bi)
                phase_d(bi)

    nc.compile()
    return nc


# ----------------------------------------------------------------------------
# host-side constants and per-core input slicing
# ----------------------------------------------------------------------------

def _rope_tables(l):
    inv_freq = 1.0 / (10000.0 ** (np.arange(0, HEAD_DIM, 2, dtype=np.float32)
                                  / HEAD_DIM))                       # [32]
    t = np.arange(l, dtype=np.float32)
    fr = t[None, :] * inv_freq[:, None]                              # [32, l]
    cos1 = np.cos(np.concatenate([fr, fr], axis=0))                  # [64, l]
    sin1 = np.sin(np.concatenate([fr, fr], axis=0))                  # [64, l]
    sinm1 = np.concatenate([-sin1[:32], sin1[32:]], axis=0)          # sign-folded
    # half-swapped so the stt source base partition matches the operand rows
    sinswap1 = np.concatenate([sinm1[32:], sinm1[:32]], axis=0)
    cos = np.tile(cos1, (HPC, 1)).astype(np.float32)                 # [128, l]
    sinswap = np.tile(sinswap1, (HPC, 1)).astype(np.float32)
    return cos, sinswap


def _mask_strip():
    # strip[r, u] = 1 iff u >= r + 384; diagonal block at offset d uses
    # cols [384-d : 896-d] so that mask[r, c] = (c >= r + d)
    r = np.arange(128)[:, None]
    u = np.arange(896)[None, :]
    return (u >= r + 384).astype(np.float32)


def core_inputs(x_bf, ln_w, ln_b, W_in, W_out, c, l=L):
    """Build the per-core input map for core c (pure numpy).

    x_bf: [T, HIDDEN] bf16 (pre-cast once by the caller)."""
    import ml_dtypes
    bf16 = ml_dtypes.bfloat16
    ln_w = np.asarray(ln_w, np.float32)
    ln_b = np.asarray(ln_b, np.float32)
    W_in = np.asarray(W_in, np.float32)
    W_out = np.asarray(W_out, np.float32)

    qc = slice(c * QS, (c + 1) * QS)
    kc = slice(HIDDEN + c * QS, HIDDEN + (c + 1) * QS)
    vc = slice(2 * HIDDEN + c * QS, 2 * HIDDEN + (c + 1) * QS)
    fc = slice(3 * HIDDEN + c * FFS, 3 * HIDDEN + (c + 1) * FFS)
    w_raw = np.concatenate(
        [W_in[:, qc], W_in[:, kc], W_in[:, vc], W_in[:, fc]], axis=1)  # [1024, 896]
    w_slice = w_raw * ln_w[:, None]
    # device layout [128, KOUT, WSL]: w_arr[p, o, f] = w_slice[o*128+p, f]
    w_arr = np.ascontiguousarray(
        w_slice.reshape(KOUT, 128, WSL).transpose(1, 0, 2).astype(bf16))
    h_bias = (ln_b @ w_raw)                                            # [896]
    hb_arr = np.ascontiguousarray(
        h_bias.reshape(WSL // 128, 128).T.astype(np.float32))          # [128, 7]
    wo_slice = np.concatenate(
        [W_out[c * QS:(c + 1) * QS, :],
         W_out[HIDDEN + c * FFS: HIDDEN + (c + 1) * FFS, :]], axis=0)  # [640, 1024]
    wo_arr = np.ascontiguousarray(
        wo_slice.reshape(WOK, 128, HIDDEN).transpose(1, 0, 2).astype(bf16))

    cos, sinm = _rope_tables(l)
    return {
        "x": x_bf,
        "w_in": w_arr,
        "w_out": wo_arr,
        "h_bias": hb_arr,
        "cos_t": np.ascontiguousarray(cos.astype(bf16)),
        "sinm_t": np.ascontiguousarray(sinm.astype(bf16)),
        "mask_t": np.ascontiguousarray(_mask_strip().astype(bf16)),
    }


# ----------------------------------------------------------------------------
# entry point
# ----------------------------------------------------------------------------

_PROG_CACHE = {}


def kernel(x, ln_w, ln_b, W_in, W_out):
    global LAST_RESULTS
    import ml_dtypes
    from concourse import bass_utils
    from concourse.bass_interp import get_hw_module

    x = np.asarray(x, np.float32)
    b, l = x.shape[0], x.shape[1]

    key = (b, l)
    if key not in _PROG_CACHE:
        _PROG_CACHE[key] = build_program(b=b, l=l, debug=False)
    nc = _PROG_CACHE[key]

    x_bf = np.ascontiguousarray(
        x.reshape(b * l, HIDDEN).astype(ml_dtypes.bfloat16))
    in_maps = [core_inputs(x_bf, ln_w, ln_b, W_in, W_out, c, l=l)
               for c in range(NCORES)]

    old_m = nc.m
    nc.m = get_hw_module(nc.m)
    try:
        res = bass_utils.run_bass_kernel_spmd(
            nc, in_maps, core_ids=list(range(NCORES)),
            trace=bool(int(__import__("os").environ.get("BASS_TRACE_RUN", "0"))))
    finally:
        nc.m = old_m
    LAST_RESULTS = res

    acc = np.zeros((b * l, HIDDEN), np.float64)
    for r in res.results:
        acc += r["out"].astype(np.float64)
    return acc.reshape(b, l, HIDDEN).astype(np.float32)
